# revision 2
# baseline (speedup 1.0000x reference)
"""Trainium2 Bass kernel for nn_DynResNet (B=256, DIM=64, K=16, L=8).

Strategy (validated numerically against the jax reference in fp64/fp32):
- Pure data parallel: 32 samples per core x 8 cores; 9 shared 4096x4096
  weights streamed from HBM as fp8e4m3 (weight rounding only affects the
  output through H=1e-3-damped updates; measured end-to-end error ~5e-5).
- The Cayley u/v updates change u and v by ~1e-7 relative (below fp32 ulp
  of u); dropping them is within ~2e-6 of the fp32 reference. Only the s
  update (s += H * u^T relu(lin) v) and Xf = u s v^T recompute remain.
- Big matmul mapping: stationary = Xf^T chunks (K=128, M=32 batch) with
  4-way column tiling over k-chunks (partition-group partial sums), moving
  = W^T tiles (128, 512) fp8. Partial sums are reduced AND transposed in
  one PE pass against a stacked-identity, which also yields dY in the
  (j-partition, i, b) layout the per-sample small matmuls consume.
"""

import numpy as np
import ml_dtypes

import concourse.bass as bass
import concourse.tile as tile
from concourse import bacc, mybir
from concourse.bass_utils import run_bass_kernel_spmd

DIM, KR, L, DD, B, NCORES = 64, 16, 8, 4096, 256, 8
BLOC = B // NCORES  # 32
H = 1e-3
F32 = mybir.dt.float32
W_DT = mybir.dt.float8e4
X_DT = mybir.dt.bfloat16
W_NP = ml_dtypes.float8_e4m3
X_NP = ml_dtypes.bfloat16

_CACHE = {}


def build_nc():
    nc = bacc.Bacc("TRN2", target_bir_lowering=False, debug=False,
                   num_devices=NCORES)

    def inp(name, shape, dt):
        return nc.dram_tensor(name, shape, dt, kind="ExternalInput").ap()

    wq = inp("wq", (L + 1, 8, 128, 32, 512), W_DT)
    u64 = inp("u64", (64, BLOC, KR), F32)
    ut16 = inp("ut16", (KR, BLOC, 64), F32)
    v64 = inp("v64", (64, BLOC, KR), F32)
    vt16 = inp("vt16", (KR, BLOC, 64), F32)
    s16 = inp("s16", (KR, BLOC, KR), F32)
    xft0 = inp("xft0", (128, 32, BLOC), X_DT)
    bt = inp("bt", (64, L + 1, 64), F32)
    e4 = inp("e4", (128, BLOC), F32)
    wct = inp("wct", (128, 32, 10), F32)
    bc1 = inp("bc1", (1, 10), F32)
    ones1 = inp("ones1", (1, BLOC), F32)

    o_transf = nc.dram_tensor("o_transf", (L + 1, 128, 32, BLOC), F32,
                              kind="ExternalOutput").ap()
    o_cls = nc.dram_tensor("o_cls", (BLOC, 10), F32,
                           kind="ExternalOutput").ap()
    o_pred = nc.dram_tensor("o_pred", (BLOC, 10), F32,
                            kind="ExternalOutput").ap()

    add, mult, mx_op = (mybir.AluOpType.add, mybir.AluOpType.mult,
                        mybir.AluOpType.max)

    with tile.TileContext(nc) as tc:
        with (
            tc.tile_pool(name="consts", bufs=1) as consts,
            tc.tile_pool(name="wpool", bufs=3) as wpool,
            tc.tile_pool(name="xft", bufs=2) as xftp,
            tc.tile_pool(name="xff", bufs=2) as xffp,
            tc.tile_pool(name="dyt", bufs=2) as dytp,
            tc.tile_pool(name="lp", bufs=3) as lpp,
            tc.tile_pool(name="work", bufs=2) as work,
            tc.tile_pool(name="pbig", bufs=2, space="PSUM") as pbig,
            tc.tile_pool(name="prd", bufs=2, space="PSUM") as prdp,
            tc.tile_pool(name="psm", bufs=2, space="PSUM") as psm,
            tc.tile_pool(name="pmt", bufs=2, space="PSUM") as pmtp,
        ):
            def cload(ap_in, shape, dt):
                t = consts.tile(shape, dt, tag=ap_in.tensor.name)
                nc.sync.dma_start(t[:], ap_in)
                return t

            u64sb = cload(u64, [64, BLOC, KR], F32)
            ut16sb = cload(ut16, [KR, BLOC, 64], F32)
            v64sb = cload(v64, [64, BLOC, KR], F32)
            vt16sb = cload(vt16, [KR, BLOC, 64], F32)
            s16sb = cload(s16, [KR, BLOC, KR], F32)
            btsb = cload(bt, [64, L + 1, 64], F32)
            e4sb = cload(e4, [128, BLOC], F32)
            wctsb = cload(wct, [128, 32, 10], F32)
            bc1sb = cload(bc1, [1, 10], F32)
            ones1sb = cload(ones1, [1, BLOC], F32)

            xft_bf = xftp.tile([128, 32, BLOC], X_DT, tag="xft")
            nc.sync.dma_start(xft_bf[:], xft0)
            xff_cur = None

            for l in range(L + 1):
                dyt = dytp.tile([64, 64, BLOC], F32, tag="dyt")
                for ncn in range(8):
                    wt = wpool.tile([128, 32, 512], W_DT, tag="wt")
                    nc.sync.dma_start(wt[:], wq[l, ncn])
                    pa = pbig.tile([128, 512], F32, tag="pa")
                    for g in range(8):
                        for j in range(4):
                            kc = 4 * g + j
                            nc.tensor.matmul(
                                pa[32 * j:32 * j + 32, :],
                                xft_bf[:, kc, :],
                                wt[:, kc, :],
                                start=(g == 0), stop=(g == 7),
                                tile_position=(0, 32 * j),
                                skip_group_check=True,
                            )
                    lp = lpp.tile([128, 512], F32, tag="lp")
                    nc.vector.tensor_copy(lp[:], pa[:])
                    for m in range(8):
                        i = ncn * 8 + m
                        pr = prdp.tile([64, BLOC], F32, tag="pr")
                        nc.tensor.matmul(pr[:], lp[:, 64 * m:64 * m + 64],
                                         e4sb[:], start=True, stop=True)
                        # dY^T slice: relu(lin + bias), bias per-partition
                        nc.vector.tensor_scalar(
                            dyt[:, i, :], pr[:], btsb[:, l, i:i + 1], 0.0,
                            add, mx_op)

                # z_u = dY @ v  per sample (stationary = dY^T strided AP)
                pz = psm.tile([64, BLOC, KR], F32, tag="sm")
                for bb in range(BLOC):
                    nc.tensor.matmul(pz[:, bb, :], dyt[:, :, bb],
                                     v64sb[:, bb, :], start=True, stop=True)
                zu = work.tile([64, BLOC, KR], F32, tag="zu")
                nc.vector.tensor_copy(zu[:], pz[:])

                # dS = u^T z_u ; s += H * dS
                pds = psm.tile([KR, BLOC, KR], F32, tag="sm")
                for bb in range(BLOC):
                    nc.tensor.matmul(pds[:, bb, :], u64sb[:, bb, :],
                                     zu[:, bb, :], start=True, stop=True)
                nc.vector.scalar_tensor_tensor(s16sb[:], pds[:], float(H),
                                               s16sb[:], mult, add)

                # r1 = s^T u^T  (16, 64) per sample
                r1 = work.tile([KR, BLOC, 64], F32, tag="r1")
                for grp in range(4):
                    p1 = psm.tile([KR, 8, 64], F32, tag="sm")
                    for bb in range(8):
                        bs = grp * 8 + bb
                        nc.tensor.matmul(p1[:, bb, :], s16sb[:, bs, :],
                                         ut16sb[:, bs, :], start=True,
                                         stop=True)
                    nc.scalar.activation(r1[:, grp * 8:grp * 8 + 8, :],
                                         p1[:],
                                         mybir.ActivationFunctionType.Copy)

                # M^T = v r1 per sample; assemble Xf^T chunks (both halves)
                xft_new = xftp.tile([128, 32, BLOC], X_DT, tag="xft")
                xff_new = xffp.tile([128, 32, BLOC], F32, tag="xff")
                for bb in range(BLOC):
                    pm = pmtp.tile([128, 32, 2], F32, tag="pm")
                    nc.tensor.matmul(pm[0:64, :, :], vt16sb[:, bb, :],
                                     r1[:, bb, :], start=True, stop=True)
                    nc.tensor.matmul(pm[64:128, :, :], vt16sb[:, bb, :],
                                     r1[:, bb, :], start=True, stop=True,
                                     tile_position=(0, 64))
                    nc.vector.tensor_copy(xft_new[0:64, :, bb],
                                          pm[0:64, :, 0])
                    nc.vector.tensor_copy(xft_new[64:128, :, bb],
                                          pm[64:128, :, 1])
                    nc.vector.tensor_copy(xff_new[0:64, :, bb],
                                          pm[0:64, :, 0])
                    nc.vector.tensor_copy(xff_new[64:128, :, bb],
                                          pm[64:128, :, 1])
                nc.sync.dma_start(o_transf[l], xff_new[:])
                xft_bf = xft_new
                xff_cur = xff_new

            # classification head
            pc = psm.tile([BLOC, 10], F32, tag="sm")
            for c in range(32):
                nc.tensor.matmul(pc[:], xff_cur[:, c, :], wctsb[:, c, :],
                                 start=(c == 0), stop=False)
            nc.tensor.matmul(pc[:], ones1sb[:], bc1sb[:], start=False,
                             stop=True)
            cls = work.tile([BLOC, 10], F32, tag="cls")
            nc.vector.tensor_copy(cls[:], pc[:])
            nc.sync.dma_start(o_cls, cls[:])

            mxt = work.tile([BLOC, 1], F32, tag="mx")
            nc.vector.tensor_reduce(mxt[:], cls[:], mybir.AxisListType.X,
                                    mx_op)
            sh = work.tile([BLOC, 10], F32, tag="sh")
            nc.vector.tensor_scalar_sub(sh[:], cls[:], mxt[:])
            ex = work.tile([BLOC, 10], F32, tag="ex")
            nc.scalar.activation(ex[:], sh[:],
                                 mybir.ActivationFunctionType.Exp)
            sm = work.tile([BLOC, 1], F32, tag="sum")
            nc.vector.tensor_reduce(sm[:], ex[:], mybir.AxisListType.X, add)
            nc.vector.reciprocal(sm[:], sm[:])
            prd = work.tile([BLOC, 10], F32, tag="pd")
            nc.vector.tensor_scalar_mul(prd[:], ex[:], sm[:])
            nc.sync.dma_start(o_pred, prd[:])

    nc.compile()
    return nc


def host_prep_shared(W0, W, b, Wc, bc):
    """Per-layer shared tensors (identical on every core)."""
    wq = np.empty((L + 1, 8, 128, 32, 512), dtype=W_NP)
    for l in range(L + 1):
        Wm = W0 if l == 0 else W[l - 1]
        # rhs tile [p, n] = Wm[n0+n, kc*128+p]  -> W^T tiles
        WT = np.ascontiguousarray(Wm.T.astype(W_NP))  # (4096 k, 4096 n)
        # (32 kc, 128 p, 8 ncn, 512 n) -> (ncn, p, kc, n)
        wq[l] = WT.reshape(32, 128, 8, 512).transpose(2, 1, 0, 3)
    bt = np.zeros((64, L + 1, 64), dtype=np.float32)
    for l in range(1, L + 1):
        bt[:, l, :] = b[l - 1].reshape(64, 64).T  # [j, i]
    e4 = np.tile(np.eye(BLOC, dtype=np.float32), (4, 1))  # (128, 32)
    wct = np.ascontiguousarray(
        Wc.T.reshape(32, 128, 10).transpose(1, 0, 2)).astype(np.float32)
    bc1 = bc.reshape(1, 10).astype(np.float32)
    ones1 = np.ones((1, BLOC), dtype=np.float32)
    return dict(wq=wq, bt=bt, e4=e4, wct=wct, bc1=bc1, ones1=ones1)


def host_prep_core(Xc):
    """Per-core tensors from this core's 32-sample X slice (32, 3, 1024)."""
    u = Xc[:, 0].reshape(BLOC, 64, 16)
    s = Xc[:, 1].reshape(BLOC, 64, 16)[:, :16, :16]
    vh = Xc[:, 2].reshape(BLOC, 16, 64)
    u64 = np.ascontiguousarray(u.transpose(1, 0, 2), dtype=np.float32)
    ut16 = np.ascontiguousarray(u.transpose(2, 0, 1), dtype=np.float32)
    v64 = np.ascontiguousarray(vh.transpose(2, 0, 1), dtype=np.float32)
    vt16 = np.ascontiguousarray(vh.transpose(1, 0, 2), dtype=np.float32)
    s16 = np.ascontiguousarray(s.transpose(1, 0, 2), dtype=np.float32)
    Xf0 = np.einsum('bik,bkl,blj->bij', u, s, vh,
                    optimize=True).reshape(BLOC, DD)
    # xft0[p, c, b] = Xf0[b, 128c + p]
    xft0 = np.ascontiguousarray(
        Xf0.T.reshape(32, 128, BLOC).transpose(1, 0, 2)).astype(X_NP)
    return dict(u64=u64, ut16=ut16, v64=v64, vt16=vt16, s16=s16, xft0=xft0)


def assemble_outputs(results):
    """results: list of 8 per-core dicts -> full outputs."""
    preds, clss, transfs = [], [], []
    for r in results:
        preds.append(r["o_pred"])
        clss.append(r["o_cls"])
        ot = r["o_transf"]  # (9, 128, 32, 32) [l, p, c, b]
        transfs.append(np.ascontiguousarray(
            ot.transpose(3, 2, 1, 0)).reshape(BLOC, DD, L + 1))
    X_predicted = np.concatenate(preds, axis=0).astype(np.float32)
    X_classified = np.concatenate(clss, axis=0).astype(np.float32)
    X_transformed = np.concatenate(transfs, axis=0).astype(np.float32)
    return X_predicted, X_classified, X_transformed


def run(X, W0, W, b, Wc, bc, **run_kwargs):
    if "nc" not in _CACHE:
        _CACHE["nc"] = build_nc()
    nc = _CACHE["nc"]
    shared = host_prep_shared(np.asarray(W0, np.float32),
                              np.asarray(W, np.float32),
                              np.asarray(b, np.float32),
                              np.asarray(Wc, np.float32),
                              np.asarray(bc, np.float32))
    X = np.asarray(X, np.float32)
    in_maps = []
    for c in range(NCORES):
        m = dict(shared)
        m.update(host_prep_core(X[c * BLOC:(c + 1) * BLOC]))
        in_maps.append(m)
    res = run_bass_kernel_spmd(nc, in_maps, core_ids=list(range(NCORES)),
                               **run_kwargs)
    return assemble_outputs(res.results), res


def kernel(X, W0, W, b, Wc, bc):
    outs, _ = run(X, W0, W, b, Wc, bc)
    return outs


# revision 5
# speedup vs baseline: 1.2950x; 1.2950x over previous
"""Trainium2 Bass kernel for nn_DynResNet (B=256, DIM=64, K=16, L=8).

Strategy (validated numerically against the jax reference in fp64/fp32):
- Pure data parallel: 32 samples per core x 8 cores; 9 shared 4096x4096
  weights streamed from HBM as fp8e4m3 (weight rounding only affects the
  output through H=1e-3-damped updates; measured end-to-end error ~5e-5).
- The Cayley u/v updates change u and v by ~1e-7 relative (below fp32 ulp
  of u); dropping them is within ~2e-6 of the fp32 reference. Only the s
  update (s += H * u^T relu(lin) v) and Xf = u s v^T recompute remain.
- Big matmul: stationary = Xf^T k-chunks (128, 32) bf16 in 4 column-tiled
  array groups, reused across all 8 output-chunk PSUM banks via
  ldweights=False follow-on matmuls; moving = W^T tiles (128, 512) fp8.
  Partition-group partial sums are reduced AND transposed in one PE pass
  against a stacked identity, yielding dY in the (j, i, b) layout the
  per-sample small matmuls consume.
"""

import numpy as np
import ml_dtypes

import concourse.bass as bass
import concourse.tile as tile
from concourse import bacc, mybir
from concourse.bass_utils import run_bass_kernel_spmd

DIM, KR, L, DD, B, NCORES = 64, 16, 8, 4096, 256, 8
BLOC = B // NCORES  # 32
H = 1e-3
F32 = mybir.dt.float32
W_DT = mybir.dt.float8e4
X_DT = mybir.dt.bfloat16
W_NP = ml_dtypes.float8_e4m3
X_NP = ml_dtypes.bfloat16

_CACHE = {}


def build_nc():
    nc = bacc.Bacc("TRN2", target_bir_lowering=False, debug=False,
                   num_devices=NCORES)

    def inp(name, shape, dt):
        return nc.dram_tensor(name, shape, dt, kind="ExternalInput").ap()

    # wq[l, r, p, j, c, n] = W_l^T[(4r+j)*128 + p, c*512 + n]
    wq = inp("wq", (L + 1, 8, 128, 4, 8, 512), W_DT)
    u64 = inp("u64", (64, BLOC, KR), F32)
    ut16 = inp("ut16", (KR, BLOC, 64), F32)
    v64 = inp("v64", (64, BLOC, KR), F32)
    vt16 = inp("vt16", (KR, BLOC, 64), F32)
    s16 = inp("s16", (KR, BLOC, KR), F32)
    xft0 = inp("xft0", (128, 32, BLOC), X_DT)
    bt = inp("bt", (64, L + 1, 64), F32)
    e4 = inp("e4", (128, BLOC), F32)
    wct = inp("wct", (128, 32, 10), F32)
    bc1 = inp("bc1", (1, 10), F32)
    ones1 = inp("ones1", (1, BLOC), F32)

    o_transf = nc.dram_tensor("o_transf", (L + 1, 128, 32, BLOC), F32,
                              kind="ExternalOutput").ap()
    o_cls = nc.dram_tensor("o_cls", (BLOC, 10), F32,
                           kind="ExternalOutput").ap()
    o_pred = nc.dram_tensor("o_pred", (BLOC, 10), F32,
                            kind="ExternalOutput").ap()

    add, mult, mx_op = (mybir.AluOpType.add, mybir.AluOpType.mult,
                        mybir.AluOpType.max)

    with tile.TileContext(nc) as tc:
        with (
            tc.tile_pool(name="consts", bufs=1) as consts,
            tc.tile_pool(name="wpool", bufs=3) as wpool,
            tc.tile_pool(name="xft", bufs=2) as xftp,
            tc.tile_pool(name="xff", bufs=2) as xffp,
            tc.tile_pool(name="dyt", bufs=2) as dytp,
            tc.tile_pool(name="lp", bufs=3) as lpp,
            tc.tile_pool(name="work", bufs=2) as work,
            tc.tile_pool(name="psum", bufs=8, space="PSUM") as psum,
        ):
            def cload(ap_in, shape, dt):
                t = consts.tile(shape, dt, tag=ap_in.tensor.name)
                nc.sync.dma_start(t[:], ap_in)
                return t

            u64sb = cload(u64, [64, BLOC, KR], F32)
            ut16sb = cload(ut16, [KR, BLOC, 64], F32)
            v64sb = cload(v64, [64, BLOC, KR], F32)
            vt16sb = cload(vt16, [KR, BLOC, 64], F32)
            s16sb = cload(s16, [KR, BLOC, KR], F32)
            btsb = cload(bt, [64, L + 1, 64], F32)
            e4sb = cload(e4, [128, BLOC], F32)
            wctsb = cload(wct, [128, 32, 10], F32)
            bc1sb = cload(bc1, [1, 10], F32)
            ones1sb = cload(ones1, [1, BLOC], F32)

            xft_bf = xftp.tile([128, 32, BLOC], X_DT, tag="xft")
            nc.sync.dma_start(xft_bf[:], xft0)
            xff_cur = None

            for l in range(L + 1):
                # ---- big matmul: all 8 n-chunk banks, k-chunk-outer ----
                pa = [psum.tile([128, 512], F32, tag="bank",
                                name=f"pa{l}_{i}")
                      for i in range(8)]
                for r in range(8):
                    wt = wpool.tile([128, 4, 8, 512], W_DT, tag="wt")
                    nc.sync.dma_start(wt[:], wq[l, r])
                    for cn in range(8):
                        for j in range(4):
                            kc = 4 * r + j
                            mm = nc.tensor.matmul(
                                pa[cn][32 * j:32 * j + 32, :],
                                xft_bf[:, kc, :],
                                wt[:, j, cn, :],
                                start=(r == 0), stop=(r == 7),
                                tile_position=(0, 32 * j),
                                skip_group_check=True,
                            )
                            if cn > 0:
                                mm.ldweights = False

                # ---- reduce partition groups + transpose + bias + relu ----
                dyt = dytp.tile([64, 64, BLOC], F32, tag="dyt")
                for cn in range(8):
                    lp = lpp.tile([128, 512], F32, tag="lp")
                    nc.vector.tensor_copy(lp[:], pa[cn][:])
                    prd = psum.tile([64, 8, BLOC], F32, tag="bank")
                    for m in range(8):
                        nc.tensor.matmul(prd[:, m, :],
                                         lp[:, 64 * m:64 * m + 64],
                                         e4sb[:], start=True, stop=True)
                    dsl = dyt[:, 8 * cn:8 * cn + 8, :]
                    bias_bc = btsb[:, l, 8 * cn:8 * cn + 8][:, :, None] \
                        .broadcast_to((64, 8, BLOC))
                    nc.vector.tensor_add(dsl, prd[:], bias_bc)
                    nc.vector.tensor_scalar_max(dsl, dsl, 0.0)

                # ---- z_u = dY v ; dS = u^T z_u ; s += H dS ----
                pz = psum.tile([64, BLOC, KR], F32, tag="bank")
                for bb in range(BLOC):
                    nc.tensor.matmul(pz[:, bb, :], dyt[:, :, bb],
                                     v64sb[:, bb, :], start=True, stop=True)
                zu = work.tile([64, BLOC, KR], F32, tag="zu")
                nc.vector.tensor_copy(zu[:], pz[:])
                pds = psum.tile([KR, BLOC, KR], F32, tag="bank")
                for bb in range(BLOC):
                    nc.tensor.matmul(pds[:, bb, :], u64sb[:, bb, :],
                                     zu[:, bb, :], start=True, stop=True)
                nc.vector.scalar_tensor_tensor(s16sb[:], pds[:], float(H),
                                               s16sb[:], mult, add)

                # ---- r1 = s^T u^T per sample ----
                r1 = work.tile([KR, BLOC, 64], F32, tag="r1")
                for grp in range(4):
                    p1 = psum.tile([KR, 8, 64], F32, tag="bank")
                    for bb in range(8):
                        bs = grp * 8 + bb
                        nc.tensor.matmul(p1[:, bb, :], s16sb[:, bs, :],
                                         ut16sb[:, bs, :], start=True,
                                         stop=True)
                    nc.scalar.activation(r1[:, grp * 8:grp * 8 + 8, :],
                                         p1[:],
                                         mybir.ActivationFunctionType.Copy)

                # ---- M^T = v r1 ; assemble Xf^T (both halves), batched ----
                xft_new = xftp.tile([128, 32, BLOC], X_DT, tag="xft")
                xff_new = xffp.tile([128, 32, BLOC], F32, tag="xff")
                for grp in range(4):
                    pm = psum.tile([128, 8, 64], F32, tag="bank")
                    for bb in range(8):
                        bs = grp * 8 + bb
                        nc.tensor.matmul(pm[0:64, bb, :],
                                         vt16sb[:, bs, :], r1[:, bs, :],
                                         start=True, stop=True)
                        nc.tensor.matmul(pm[64:128, bb, :],
                                         vt16sb[:, bs, :], r1[:, bs, :],
                                         start=True, stop=True,
                                         tile_position=(0, 64))
                    bsl = slice(8 * grp, 8 * grp + 8)
                    pm_even = pm[0:64, :, 0:64:2].transpose([0, 2, 1])
                    pm_odd = pm[64:128, :, 1:64:2].transpose([0, 2, 1])
                    nc.vector.tensor_copy(xft_new[0:64, :, bsl], pm_even)
                    nc.vector.tensor_copy(xft_new[64:128, :, bsl], pm_odd)
                    nc.vector.tensor_copy(xff_new[0:64, :, bsl], pm_even)
                    nc.vector.tensor_copy(xff_new[64:128, :, bsl], pm_odd)
                nc.sync.dma_start(o_transf[l], xff_new[:])
                xft_bf = xft_new
                xff_cur = xff_new

            # ---- classification head + softmax ----
            pc = psum.tile([BLOC, 10], F32, tag="bank")
            for c in range(32):
                nc.tensor.matmul(pc[:], xff_cur[:, c, :], wctsb[:, c, :],
                                 start=(c == 0), stop=False)
            nc.tensor.matmul(pc[:], ones1sb[:], bc1sb[:], start=False,
                             stop=True)
            cls = work.tile([BLOC, 10], F32, tag="cls")
            nc.vector.tensor_copy(cls[:], pc[:])
            nc.sync.dma_start(o_cls, cls[:])

            mxt = work.tile([BLOC, 1], F32, tag="mx")
            nc.vector.tensor_reduce(mxt[:], cls[:], mybir.AxisListType.X,
                                    mx_op)
            sh = work.tile([BLOC, 10], F32, tag="sh")
            nc.vector.tensor_scalar_sub(sh[:], cls[:], mxt[:])
            ex = work.tile([BLOC, 10], F32, tag="ex")
            nc.scalar.activation(ex[:], sh[:],
                                 mybir.ActivationFunctionType.Exp)
            sm = work.tile([BLOC, 1], F32, tag="sum")
            nc.vector.tensor_reduce(sm[:], ex[:], mybir.AxisListType.X, add)
            nc.vector.reciprocal(sm[:], sm[:])
            prd_t = work.tile([BLOC, 10], F32, tag="pd")
            nc.vector.tensor_scalar_mul(prd_t[:], ex[:], sm[:])
            nc.sync.dma_start(o_pred, prd_t[:])

    nc.compile()
    return nc


def host_prep_shared(W0, W, b, Wc, bc):
    """Per-layer shared tensors (identical on every core)."""
    wq = np.empty((L + 1, 8, 128, 4, 8, 512), dtype=W_NP)
    for l in range(L + 1):
        Wm = W0 if l == 0 else W[l - 1]
        WT = np.ascontiguousarray(Wm.T).astype(W_NP)  # (4096 k, 4096 n)
        # (8r, 4j, 128p, 8c, 512n) -> (r, p, j, c, n)
        wq[l] = WT.reshape(8, 4, 128, 8, 512).transpose(0, 2, 1, 3, 4)
    bt = np.zeros((64, L + 1, 64), dtype=np.float32)
    for l in range(1, L + 1):
        bt[:, l, :] = b[l - 1].reshape(64, 64).T  # [j, i]
    e4 = np.tile(np.eye(BLOC, dtype=np.float32), (4, 1))  # (128, 32)
    wct = np.ascontiguousarray(
        Wc.T.reshape(32, 128, 10).transpose(1, 0, 2)).astype(np.float32)
    bc1 = bc.reshape(1, 10).astype(np.float32)
    ones1 = np.ones((1, BLOC), dtype=np.float32)
    return dict(wq=wq, bt=bt, e4=e4, wct=wct, bc1=bc1, ones1=ones1)


def host_prep_core(Xc):
    """Per-core tensors from this core's 32-sample X slice (32, 3, 1024)."""
    u = Xc[:, 0].reshape(BLOC, 64, 16)
    s = Xc[:, 1].reshape(BLOC, 64, 16)[:, :16, :16]
    vh = Xc[:, 2].reshape(BLOC, 16, 64)
    u64 = np.ascontiguousarray(u.transpose(1, 0, 2), dtype=np.float32)
    ut16 = np.ascontiguousarray(u.transpose(2, 0, 1), dtype=np.float32)
    v64 = np.ascontiguousarray(vh.transpose(2, 0, 1), dtype=np.float32)
    vt16 = np.ascontiguousarray(vh.transpose(1, 0, 2), dtype=np.float32)
    s16 = np.ascontiguousarray(s.transpose(1, 0, 2), dtype=np.float32)
    Xf0 = np.einsum('bik,bkl,blj->bij', u, s, vh,
                    optimize=True).reshape(BLOC, DD)
    # xft0[p, c, b] = Xf0[b, 128c + p]
    xft0 = np.ascontiguousarray(
        Xf0.T.reshape(32, 128, BLOC).transpose(1, 0, 2)).astype(X_NP)
    return dict(u64=u64, ut16=ut16, v64=v64, vt16=vt16, s16=s16, xft0=xft0)


def assemble_outputs(results):
    """results: list of 8 per-core dicts -> full outputs."""
    preds, clss, transfs = [], [], []
    for r in results:
        preds.append(r["o_pred"])
        clss.append(r["o_cls"])
        ot = r["o_transf"]  # (9, 128, 32, 32) [l, p, c, b]
        transfs.append(np.ascontiguousarray(
            ot.transpose(3, 2, 1, 0)).reshape(BLOC, DD, L + 1))
    X_predicted = np.concatenate(preds, axis=0).astype(np.float32)
    X_classified = np.concatenate(clss, axis=0).astype(np.float32)
    X_transformed = np.concatenate(transfs, axis=0).astype(np.float32)
    return X_predicted, X_classified, X_transformed


def run(X, W0, W, b, Wc, bc, **run_kwargs):
    if "nc" not in _CACHE:
        _CACHE["nc"] = build_nc()
    nc = _CACHE["nc"]
    shared = host_prep_shared(np.asarray(W0, np.float32),
                              np.asarray(W, np.float32),
                              np.asarray(b, np.float32),
                              np.asarray(Wc, np.float32),
                              np.asarray(bc, np.float32))
    X = np.asarray(X, np.float32)
    in_maps = []
    for c in range(NCORES):
        m = dict(shared)
        m.update(host_prep_core(X[c * BLOC:(c + 1) * BLOC]))
        in_maps.append(m)
    res = run_bass_kernel_spmd(nc, in_maps, core_ids=list(range(NCORES)),
                               **run_kwargs)
    return assemble_outputs(res.results), res


def kernel(X, W0, W, b, Wc, bc):
    outs, _ = run(X, W0, W, b, Wc, bc)
    return outs


# revision 9
# speedup vs baseline: 1.4323x; 1.1060x over previous
"""Trainium2 Bass kernel for nn_DynResNet (B=256, DIM=64, K=16, L=8).

Strategy (validated numerically against the jax reference in fp64/fp32):
- Pure data parallel: 32 samples per core x 8 cores; 9 shared 4096x4096
  weights streamed from HBM as fp8e4m3 (weight rounding only affects the
  output through H=1e-3-damped updates; measured end-to-end error ~5e-5).
- The Cayley u/v updates change u and v by ~1e-7 relative (below fp32 ulp
  of u); dropping them is within ~2e-6 of the fp32 reference. Only the s
  update (s += H * u^T relu(lin) v) and Xf = u s v^T recompute remain.
- Big matmul: stationary = Xf^T k-chunks (128, 32) bf16 in 4 column-tiled
  array groups, reused across all 8 output-chunk PSUM banks via
  ldweights=False follow-on matmuls; moving = W^T tiles (128, 512) fp8.
  Partition-group partial sums are reduced AND transposed in one PE pass
  against a stacked identity, yielding dY in the (j, i, b) layout the
  per-sample small matmuls consume.
"""

import numpy as np
import ml_dtypes

import concourse.bass as bass
import concourse.tile as tile
from concourse import bacc, mybir
from concourse.bass_utils import run_bass_kernel_spmd

DIM, KR, L, DD, B, NCORES = 64, 16, 8, 4096, 256, 8
BLOC = B // NCORES  # 32
H = 1e-3
F32 = mybir.dt.float32
W_DT = mybir.dt.float8e4
X_DT = mybir.dt.bfloat16
W_NP = ml_dtypes.float8_e4m3
X_NP = ml_dtypes.bfloat16

_CACHE = {}
MT_ROW_TILED = True


def build_nc():
    nc = bacc.Bacc("TRN2", target_bir_lowering=False, debug=False,
                   num_devices=NCORES)

    def inp(name, shape, dt):
        return nc.dram_tensor(name, shape, dt, kind="ExternalInput").ap()

    # wq[l, r, p, j, c, n] = W_l^T[(4r+j)*128 + p, c*512 + n]
    wq = inp("wq", (L + 1, 8, 128, 4, 8, 512), W_DT)
    u64 = inp("u64", (64, BLOC, KR), X_DT)
    ut16 = inp("ut16", (KR, BLOC, 64), F32)
    v64 = inp("v64", (64, BLOC, KR), X_DT)
    vt16q = inp("vt16q", (128, BLOC, 64), F32)
    s16 = inp("s16", (KR, BLOC, KR), F32)
    xft0 = inp("xft0", (128, 32, BLOC), X_DT)
    bt = inp("bt", (64, L + 1, 64), F32)
    e4 = inp("e4", (128, BLOC), X_DT)
    wct = inp("wct", (128, 32, 10), F32)
    bc1 = inp("bc1", (1, 10), F32)
    ones1 = inp("ones1", (1, BLOC), F32)

    o_transf = nc.dram_tensor("o_transf", (L + 1, 128, 32, BLOC), F32,
                              kind="ExternalOutput").ap()
    o_cls = nc.dram_tensor("o_cls", (BLOC, 10), F32,
                           kind="ExternalOutput").ap()
    o_pred = nc.dram_tensor("o_pred", (BLOC, 10), F32,
                            kind="ExternalOutput").ap()

    add, mult, mx_op = (mybir.AluOpType.add, mybir.AluOpType.mult,
                        mybir.AluOpType.max)

    with tile.TileContext(nc) as tc:
        with (
            tc.tile_pool(name="consts", bufs=1) as consts,
            tc.tile_pool(name="wpool", bufs=3) as wpool,
            tc.tile_pool(name="xft", bufs=2) as xftp,
            tc.tile_pool(name="xff", bufs=2) as xffp,
            tc.tile_pool(name="dyt", bufs=2) as dytp,
            tc.tile_pool(name="lp", bufs=3) as lpp,
            tc.tile_pool(name="work", bufs=2) as work,
            tc.tile_pool(name="psum", bufs=8, space="PSUM") as psum,
        ):
            def cload(ap_in, shape, dt):
                t = consts.tile(shape, dt, tag=ap_in.tensor.name)
                nc.sync.dma_start(t[:], ap_in)
                return t

            u64sb = cload(u64, [64, BLOC, KR], X_DT)
            ut16sb = cload(ut16, [KR, BLOC, 64], F32)
            v64sb = cload(v64, [64, BLOC, KR], X_DT)
            vt16qsb = cload(vt16q, [128, BLOC, 64], F32)
            s16sb = cload(s16, [KR, BLOC, KR], F32)
            btsb = cload(bt, [64, L + 1, 64], F32)
            e4sb = cload(e4, [128, BLOC], X_DT)
            wctsb = cload(wct, [128, 32, 10], F32)
            bc1sb = cload(bc1, [1, 10], F32)
            ones1sb = cload(ones1, [1, BLOC], F32)

            xft_bf = xftp.tile([128, 32, BLOC], X_DT, tag="xft")
            nc.sync.dma_start(xft_bf[:], xft0)
            xff_cur = None

            for l in range(L + 1):
                # ---- big matmul: all 8 n-chunk banks, k-chunk-outer ----
                pa = [psum.tile([128, 512], F32, tag="bank",
                                name=f"pa{l}_{i}")
                      for i in range(8)]
                for r in range(8):
                    wt = wpool.tile([128, 4, 8, 512], W_DT, tag="wt")
                    nc.sync.dma_start(wt[:], wq[l, r])
                    for cn in range(8):
                        for j in range(4):
                            kc = 4 * r + j
                            mm = nc.tensor.matmul(
                                pa[cn][32 * j:32 * j + 32, :],
                                xft_bf[:, kc, :],
                                wt[:, j, cn, :],
                                start=(r == 0), stop=(r == 7),
                                tile_position=(0, 32 * j),
                                skip_group_check=True,
                            )
                            if cn > 0:
                                mm.ldweights = False

                # ---- reduce partition groups + transpose + bias + relu ----
                dyt = dytp.tile([64, 64, BLOC], X_DT, tag="dyt")
                for cn in range(8):
                    lp = lpp.tile([128, 512], X_DT, tag="lp")
                    nc.vector.tensor_copy(lp[:], pa[cn][:])
                    prd = psum.tile([64, 8, BLOC], F32, tag="bank")
                    for m in range(8):
                        nc.tensor.matmul(prd[:, m, :],
                                         lp[:, 64 * m:64 * m + 64],
                                         e4sb[:], start=True, stop=True)
                    dsl = dyt[:, 8 * cn:8 * cn + 8, :]
                    bias_bc = btsb[:, l, 8 * cn:8 * cn + 8][:, :, None] \
                        .broadcast_to((64, 8, BLOC))
                    nc.vector.tensor_add(dsl, prd[:], bias_bc)
                    nc.vector.tensor_scalar_max(dsl, dsl, 0.0)

                # ---- z_u = dY v ; dS = u^T z_u ; s += H dS ----
                pz = psum.tile([64, BLOC, KR], F32, tag="bank")
                for bb in range(BLOC):
                    nc.tensor.matmul(pz[:, bb, :], dyt[:, :, bb],
                                     v64sb[:, bb, :], start=True, stop=True)
                zu = work.tile([64, BLOC, KR], X_DT, tag="zu")
                nc.vector.tensor_copy(zu[:], pz[:])
                pds = psum.tile([KR, BLOC, KR], F32, tag="bank")
                for bb in range(BLOC):
                    nc.tensor.matmul(pds[:, bb, :], u64sb[:, bb, :],
                                     zu[:, bb, :], start=True, stop=True)
                nc.vector.scalar_tensor_tensor(s16sb[:], pds[:], float(H),
                                               s16sb[:], mult, add)

                # ---- r1 = s^T u^T per sample ----
                r1 = work.tile([KR, BLOC, 64], F32, tag="r1")
                for grp in range(4):
                    p1 = psum.tile([KR, 8, 64], F32, tag="bank")
                    for bb in range(8):
                        bs = grp * 8 + bb
                        nc.tensor.matmul(p1[:, bb, :], s16sb[:, bs, :],
                                         ut16sb[:, bs, :], start=True,
                                         stop=True)
                    nc.scalar.activation(r1[:, grp * 8:grp * 8 + 8, :],
                                         p1[:],
                                         mybir.ActivationFunctionType.Copy)

                # ---- M^T = v r1 ; assemble Xf^T (both halves), batched ----
                xft_new = xftp.tile([128, 32, BLOC], X_DT, tag="xft")
                xff_new = xffp.tile([128, 32, BLOC], F32, tag="xff")
                for grp in range(4):
                    pm = psum.tile([128, 8, 64], F32, tag="bank")
                    for bb in range(8):
                        bs = grp * 8 + bb
                        vq = vt16qsb[0:16, bs, :]
                        rq = r1[:, bs, :]
                        nc.tensor.matmul(pm[0:64, bb, :], vq, rq,
                                         start=True, stop=True)
                        nc.tensor.matmul(pm[64:128, bb, :], vq, rq,
                                         start=True, stop=True,
                                         tile_position=(0, 64))
                    bsl = slice(8 * grp, 8 * grp + 8)
                    pm_even = pm[0:64, :, 0:64:2].transpose([0, 2, 1])
                    pm_odd = pm[64:128, :, 1:64:2].transpose([0, 2, 1])
                    nc.vector.tensor_copy(xft_new[0:64, :, bsl], pm_even)
                    nc.vector.tensor_copy(xft_new[64:128, :, bsl], pm_odd)
                    nc.vector.tensor_copy(xff_new[0:64, :, bsl], pm_even)
                    nc.vector.tensor_copy(xff_new[64:128, :, bsl], pm_odd)
                nc.sync.dma_start(o_transf[l], xff_new[:])
                xft_bf = xft_new
                xff_cur = xff_new

            # ---- classification head + softmax ----
            pc = psum.tile([BLOC, 10], F32, tag="bank")
            for c in range(32):
                nc.tensor.matmul(pc[:], xff_cur[:, c, :], wctsb[:, c, :],
                                 start=(c == 0), stop=False)
            nc.tensor.matmul(pc[:], ones1sb[:], bc1sb[:], start=False,
                             stop=True)
            cls = work.tile([BLOC, 10], F32, tag="cls")
            nc.vector.tensor_copy(cls[:], pc[:])
            nc.sync.dma_start(o_cls, cls[:])

            mxt = work.tile([BLOC, 1], F32, tag="mx")
            nc.vector.tensor_reduce(mxt[:], cls[:], mybir.AxisListType.X,
                                    mx_op)
            sh = work.tile([BLOC, 10], F32, tag="sh")
            nc.vector.tensor_scalar_sub(sh[:], cls[:], mxt[:])
            ex = work.tile([BLOC, 10], F32, tag="ex")
            nc.scalar.activation(ex[:], sh[:],
                                 mybir.ActivationFunctionType.Exp)
            sm = work.tile([BLOC, 1], F32, tag="sum")
            nc.vector.tensor_reduce(sm[:], ex[:], mybir.AxisListType.X, add)
            nc.vector.reciprocal(sm[:], sm[:])
            prd_t = work.tile([BLOC, 10], F32, tag="pd")
            nc.vector.tensor_scalar_mul(prd_t[:], ex[:], sm[:])
            nc.sync.dma_start(o_pred, prd_t[:])

    nc.compile()
    return nc


def host_prep_shared(W0, W, b, Wc, bc):
    """Per-layer shared tensors (identical on every core)."""
    wq = np.empty((L + 1, 8, 128, 4, 8, 512), dtype=W_NP)
    for l in range(L + 1):
        Wm = W0 if l == 0 else W[l - 1]
        WT = np.ascontiguousarray(Wm.T).astype(W_NP)  # (4096 k, 4096 n)
        # (8r, 4j, 128p, 8c, 512n) -> (r, p, j, c, n)
        wq[l] = WT.reshape(8, 4, 128, 8, 512).transpose(0, 2, 1, 3, 4)
    bt = np.zeros((64, L + 1, 64), dtype=np.float32)
    for l in range(1, L + 1):
        bt[:, l, :] = b[l - 1].reshape(64, 64).T  # [j, i]
    e4 = np.tile(np.eye(BLOC), (4, 1)).astype(X_NP)  # (128, 32)
    wct = np.ascontiguousarray(
        Wc.T.reshape(32, 128, 10).transpose(1, 0, 2)).astype(np.float32)
    bc1 = bc.reshape(1, 10).astype(np.float32)
    ones1 = np.ones((1, BLOC), dtype=np.float32)
    return dict(wq=wq, bt=bt, e4=e4, wct=wct, bc1=bc1, ones1=ones1)


def host_prep_core(Xc):
    """Per-core tensors from this core's 32-sample X slice (32, 3, 1024)."""
    u = Xc[:, 0].reshape(BLOC, 64, 16)
    s = Xc[:, 1].reshape(BLOC, 64, 16)[:, :16, :16]
    vh = Xc[:, 2].reshape(BLOC, 16, 64)
    u64 = np.ascontiguousarray(u.transpose(1, 0, 2)).astype(X_NP)
    ut16 = np.ascontiguousarray(u.transpose(2, 0, 1), dtype=np.float32)
    v64 = np.ascontiguousarray(vh.transpose(2, 0, 1)).astype(X_NP)
    vt16 = np.ascontiguousarray(vh.transpose(1, 0, 2), dtype=np.float32)
    vt16q = np.zeros((128, BLOC, 64), dtype=np.float32)
    for q in range(4):
        vt16q[32 * q:32 * q + 16] = vt16
    s16 = np.ascontiguousarray(s.transpose(1, 0, 2), dtype=np.float32)
    Xf0 = np.einsum('bik,bkl,blj->bij', u, s, vh,
                    optimize=True).reshape(BLOC, DD)
    # xft0[p, c, b] = Xf0[b, 128c + p]
    xft0 = np.ascontiguousarray(
        Xf0.T.reshape(32, 128, BLOC).transpose(1, 0, 2)).astype(X_NP)
    return dict(u64=u64, ut16=ut16, v64=v64, vt16q=vt16q,
                s16=s16, xft0=xft0)


def assemble_outputs(results):
    """results: list of 8 per-core dicts -> full outputs."""
    preds, clss, transfs = [], [], []
    for r in results:
        preds.append(r["o_pred"])
        clss.append(r["o_cls"])
        ot = r["o_transf"]  # (9, 128, 32, 32) [l, p, c, b]
        transfs.append(np.ascontiguousarray(
            ot.transpose(3, 2, 1, 0)).reshape(BLOC, DD, L + 1))
    X_predicted = np.concatenate(preds, axis=0).astype(np.float32)
    X_classified = np.concatenate(clss, axis=0).astype(np.float32)
    X_transformed = np.concatenate(transfs, axis=0).astype(np.float32)
    return X_predicted, X_classified, X_transformed


def run(X, W0, W, b, Wc, bc, **run_kwargs):
    if "nc" not in _CACHE:
        _CACHE["nc"] = build_nc()
    nc = _CACHE["nc"]
    shared = host_prep_shared(np.asarray(W0, np.float32),
                              np.asarray(W, np.float32),
                              np.asarray(b, np.float32),
                              np.asarray(Wc, np.float32),
                              np.asarray(bc, np.float32))
    X = np.asarray(X, np.float32)
    in_maps = []
    for c in range(NCORES):
        m = dict(shared)
        m.update(host_prep_core(X[c * BLOC:(c + 1) * BLOC]))
        in_maps.append(m)
    res = run_bass_kernel_spmd(nc, in_maps, core_ids=list(range(NCORES)),
                               **run_kwargs)
    return assemble_outputs(res.results), res


def kernel(X, W0, W, b, Wc, bc):
    outs, _ = run(X, W0, W, b, Wc, bc)
    return outs


# revision 10
# speedup vs baseline: 1.4709x; 1.0270x over previous
"""Trainium2 Bass kernel for nn_DynResNet (B=256, DIM=64, K=16, L=8).

Strategy (validated numerically against the jax reference in fp64/fp32):
- Pure data parallel: 32 samples per core x 8 cores; 9 shared 4096x4096
  weights streamed from HBM as fp8e4m3 (weight rounding only affects the
  output through H=1e-3-damped updates; measured end-to-end error ~5e-5).
- The Cayley u/v updates change u and v by ~1e-7 relative (below fp32 ulp
  of u); dropping them is within ~2e-6 of the fp32 reference. Only the s
  update (s += H * u^T relu(lin) v) and Xf = u s v^T recompute remain.
- Big matmul: stationary = Xf^T k-chunks (128, 32) bf16 in 4 column-tiled
  array groups, reused across all 8 output-chunk PSUM banks via
  ldweights=False follow-on matmuls; moving = W^T tiles (128, 512) fp8.
  Partition-group partial sums are reduced AND transposed in one PE pass
  against a stacked identity, yielding dY in the (j, i, b) layout the
  per-sample small matmuls consume.
"""

import numpy as np
import ml_dtypes

import concourse.bass as bass
import concourse.tile as tile
from concourse import bacc, mybir
from concourse.bass_utils import run_bass_kernel_spmd

DIM, KR, L, DD, B, NCORES = 64, 16, 8, 4096, 256, 8
BLOC = B // NCORES  # 32
H = 1e-3
F32 = mybir.dt.float32
W_DT = mybir.dt.float8e4
X_DT = mybir.dt.bfloat16
W_NP = ml_dtypes.float8_e4m3
X_NP = ml_dtypes.bfloat16

_CACHE = {}
MT_ROW_TILED = True


def build_nc():
    nc = bacc.Bacc("TRN2", target_bir_lowering=False, debug=False,
                   num_devices=NCORES)

    def inp(name, shape, dt):
        return nc.dram_tensor(name, shape, dt, kind="ExternalInput").ap()

    # wq[l, r, p, j, c, n] = W_l^T[(4r+j)*128 + p, c*512 + n]
    wq = inp("wq", (L + 1, 8, 128, 4, 8, 512), W_DT)
    u64 = inp("u64", (64, BLOC, KR), X_DT)
    ut16 = inp("ut16", (KR, BLOC, 64), X_DT)
    v64 = inp("v64", (64, BLOC, KR), X_DT)
    vt16q = inp("vt16q", (16, BLOC, 64), X_DT)
    xft0 = inp("xft0", (128, 32, BLOC), X_DT)
    xff0 = inp("xff0", (128, 32, BLOC), F32)
    bt = inp("bt", (64, L + 1, 64), F32)
    e4 = inp("e4", (128, BLOC), X_DT)
    wct = inp("wct", (128, 32, 10), F32)
    bc1 = inp("bc1", (1, 10), F32)
    ones1 = inp("ones1", (1, BLOC), F32)

    o_transf = nc.dram_tensor("o_transf", (L + 1, 128, 32, BLOC), F32,
                              kind="ExternalOutput").ap()
    o_cls = nc.dram_tensor("o_cls", (BLOC, 10), F32,
                           kind="ExternalOutput").ap()
    o_pred = nc.dram_tensor("o_pred", (BLOC, 10), F32,
                            kind="ExternalOutput").ap()

    add, mult, mx_op = (mybir.AluOpType.add, mybir.AluOpType.mult,
                        mybir.AluOpType.max)

    with tile.TileContext(nc) as tc:
        with (
            tc.tile_pool(name="consts", bufs=1) as consts,
            tc.tile_pool(name="wpool", bufs=3) as wpool,
            tc.tile_pool(name="xft", bufs=2) as xftp,
            tc.tile_pool(name="xff", bufs=2) as xffp,
            tc.tile_pool(name="dyt", bufs=2) as dytp,
            tc.tile_pool(name="lp", bufs=3) as lpp,
            tc.tile_pool(name="work", bufs=2) as work,
            tc.tile_pool(name="psum", bufs=8, space="PSUM") as psum,
        ):
            def cload(ap_in, shape, dt):
                t = consts.tile(shape, dt, tag=ap_in.tensor.name)
                nc.sync.dma_start(t[:], ap_in)
                return t

            u64sb = cload(u64, [64, BLOC, KR], X_DT)
            ut16sb = cload(ut16, [KR, BLOC, 64], X_DT)
            v64sb = cload(v64, [64, BLOC, KR], X_DT)
            vt16qsb = cload(vt16q, [16, BLOC, 64], X_DT)
            btsb = cload(bt, [64, L + 1, 64], F32)
            e4sb = cload(e4, [128, BLOC], X_DT)
            wctsb = cload(wct, [128, 32, 10], F32)
            bc1sb = cload(bc1, [1, 10], F32)
            ones1sb = cload(ones1, [1, BLOC], F32)

            xft_bf = xftp.tile([128, 32, BLOC], X_DT, tag="xft")
            nc.sync.dma_start(xft_bf[:], xft0)
            xff_cur = xffp.tile([128, 32, BLOC], F32, tag="xff")
            nc.sync.dma_start(xff_cur[:], xff0)

            for l in range(L + 1):
                # ---- big matmul: all 8 n-chunk banks, k-chunk-outer ----
                pa = [psum.tile([128, 512], F32, tag="bank",
                                name=f"pa{l}_{i}")
                      for i in range(8)]
                for r in range(8):
                    wt = wpool.tile([128, 4, 8, 512], W_DT, tag="wt")
                    nc.sync.dma_start(wt[:], wq[l, r])
                    for cn in range(8):
                        for j in range(4):
                            kc = 4 * r + j
                            mm = nc.tensor.matmul(
                                pa[cn][32 * j:32 * j + 32, :],
                                xft_bf[:, kc, :],
                                wt[:, j, cn, :],
                                start=(r == 0), stop=(r == 7),
                                tile_position=(0, 32 * j),
                                skip_group_check=True,
                            )
                            if cn > 0:
                                mm.ldweights = False

                # ---- reduce partition groups + transpose + bias + relu ----
                dyt = dytp.tile([64, 64, BLOC], X_DT, tag="dyt")
                for cn in range(8):
                    lp = lpp.tile([128, 512], X_DT, tag="lp")
                    nc.vector.tensor_copy(lp[:], pa[cn][:])
                    prd = psum.tile([64, 8, BLOC], F32, tag="bank")
                    for m in range(8):
                        nc.tensor.matmul(prd[:, m, :],
                                         lp[:, 64 * m:64 * m + 64],
                                         e4sb[:], start=True, stop=True)
                    dsl = dyt[:, 8 * cn:8 * cn + 8, :]
                    bias_bc = btsb[:, l, 8 * cn:8 * cn + 8][:, :, None] \
                        .broadcast_to((64, 8, BLOC))
                    nc.vector.tensor_add(dsl, prd[:], bias_bc)
                    nc.vector.tensor_scalar_max(dsl, dsl, 0.0)

                # ---- z_u = dY v ; dS = u^T z_u ; s += H dS ----
                pz = psum.tile([64, BLOC, KR], F32, tag="bank")
                for bb in range(BLOC):
                    nc.tensor.matmul(pz[:, bb, :], dyt[:, :, bb],
                                     v64sb[:, bb, :], start=True, stop=True)
                zu = work.tile([64, BLOC, KR], X_DT, tag="zu")
                nc.vector.tensor_copy(zu[:], pz[:])
                pds = psum.tile([KR, BLOC, KR], F32, tag="bank")
                for bb in range(BLOC):
                    nc.tensor.matmul(pds[:, bb, :], u64sb[:, bb, :],
                                     zu[:, bb, :], start=True, stop=True)
                dsb = work.tile([KR, BLOC, KR], X_DT, tag="dsb")
                nc.vector.tensor_scalar_mul(dsb[:], pds[:], float(H))

                # ---- r1 = (H dS)^T u^T per sample ----
                r1 = work.tile([KR, BLOC, 64], X_DT, tag="r1")
                for grp in range(4):
                    p1 = psum.tile([KR, 8, 64], F32, tag="bank")
                    for bb in range(8):
                        bs = grp * 8 + bb
                        nc.tensor.matmul(p1[:, bb, :], dsb[:, bs, :],
                                         ut16sb[:, bs, :], start=True,
                                         stop=True)
                    nc.scalar.activation(r1[:, grp * 8:grp * 8 + 8, :],
                                         p1[:],
                                         mybir.ActivationFunctionType.Copy)

                # ---- dM^T = v r1 ; Xf += dM (both halves), batched ----
                xft_new = xftp.tile([128, 32, BLOC], X_DT, tag="xft")
                xff_new = xffp.tile([128, 32, BLOC], F32, tag="xff")
                for grp in range(4):
                    pm = psum.tile([128, 8, 64], F32, tag="bank")
                    for bb in range(8):
                        bs = grp * 8 + bb
                        vq = vt16qsb[:, bs, :]
                        rq = r1[:, bs, :]
                        nc.tensor.matmul(pm[0:64, bb, :], vq, rq,
                                         start=True, stop=True)
                        nc.tensor.matmul(pm[64:128, bb, :], vq, rq,
                                         start=True, stop=True,
                                         tile_position=(0, 64))
                    bsl = slice(8 * grp, 8 * grp + 8)
                    pm_even = pm[0:64, :, 0:64:2].transpose([0, 2, 1])
                    pm_odd = pm[64:128, :, 1:64:2].transpose([0, 2, 1])
                    nc.vector.tensor_add(xff_new[0:64, :, bsl],
                                         xff_cur[0:64, :, bsl], pm_even)
                    nc.vector.tensor_add(xff_new[64:128, :, bsl],
                                         xff_cur[64:128, :, bsl], pm_odd)
                    nc.vector.tensor_copy(xft_new[0:64, :, bsl],
                                          xff_new[0:64, :, bsl])
                    nc.vector.tensor_copy(xft_new[64:128, :, bsl],
                                          xff_new[64:128, :, bsl])
                nc.sync.dma_start(o_transf[l], xff_new[:])
                xft_bf = xft_new
                xff_cur = xff_new

            # ---- classification head + softmax ----
            pc = psum.tile([BLOC, 10], F32, tag="bank")
            for c in range(32):
                nc.tensor.matmul(pc[:], xff_cur[:, c, :], wctsb[:, c, :],
                                 start=(c == 0), stop=False)
            nc.tensor.matmul(pc[:], ones1sb[:], bc1sb[:], start=False,
                             stop=True)
            cls = work.tile([BLOC, 10], F32, tag="cls")
            nc.vector.tensor_copy(cls[:], pc[:])
            nc.sync.dma_start(o_cls, cls[:])

            mxt = work.tile([BLOC, 1], F32, tag="mx")
            nc.vector.tensor_reduce(mxt[:], cls[:], mybir.AxisListType.X,
                                    mx_op)
            sh = work.tile([BLOC, 10], F32, tag="sh")
            nc.vector.tensor_scalar_sub(sh[:], cls[:], mxt[:])
            ex = work.tile([BLOC, 10], F32, tag="ex")
            nc.scalar.activation(ex[:], sh[:],
                                 mybir.ActivationFunctionType.Exp)
            sm = work.tile([BLOC, 1], F32, tag="sum")
            nc.vector.tensor_reduce(sm[:], ex[:], mybir.AxisListType.X, add)
            nc.vector.reciprocal(sm[:], sm[:])
            prd_t = work.tile([BLOC, 10], F32, tag="pd")
            nc.vector.tensor_scalar_mul(prd_t[:], ex[:], sm[:])
            nc.sync.dma_start(o_pred, prd_t[:])

    nc.compile()
    return nc


def host_prep_shared(W0, W, b, Wc, bc):
    """Per-layer shared tensors (identical on every core)."""
    wq = np.empty((L + 1, 8, 128, 4, 8, 512), dtype=W_NP)
    for l in range(L + 1):
        Wm = W0 if l == 0 else W[l - 1]
        WT = np.ascontiguousarray(Wm.T).astype(W_NP)  # (4096 k, 4096 n)
        # (8r, 4j, 128p, 8c, 512n) -> (r, p, j, c, n)
        wq[l] = WT.reshape(8, 4, 128, 8, 512).transpose(0, 2, 1, 3, 4)
    bt = np.zeros((64, L + 1, 64), dtype=np.float32)
    for l in range(1, L + 1):
        bt[:, l, :] = b[l - 1].reshape(64, 64).T  # [j, i]
    e4 = np.tile(np.eye(BLOC), (4, 1)).astype(X_NP)  # (128, 32)
    wct = np.ascontiguousarray(
        Wc.T.reshape(32, 128, 10).transpose(1, 0, 2)).astype(np.float32)
    bc1 = bc.reshape(1, 10).astype(np.float32)
    ones1 = np.ones((1, BLOC), dtype=np.float32)
    return dict(wq=wq, bt=bt, e4=e4, wct=wct, bc1=bc1, ones1=ones1)


def host_prep_core(Xc):
    """Per-core tensors from this core's 32-sample X slice (32, 3, 1024)."""
    u = Xc[:, 0].reshape(BLOC, 64, 16)
    s = Xc[:, 1].reshape(BLOC, 64, 16)[:, :16, :16]
    vh = Xc[:, 2].reshape(BLOC, 16, 64)
    u64 = np.ascontiguousarray(u.transpose(1, 0, 2)).astype(X_NP)
    ut16 = np.ascontiguousarray(u.transpose(2, 0, 1)).astype(X_NP)
    v64 = np.ascontiguousarray(vh.transpose(2, 0, 1)).astype(X_NP)
    vt16q = np.ascontiguousarray(vh.transpose(1, 0, 2)).astype(X_NP)
    s16 = np.ascontiguousarray(s.transpose(1, 0, 2), dtype=np.float32)
    Xf0 = np.einsum('bik,bkl,blj->bij', u, s, vh,
                    optimize=True).reshape(BLOC, DD)
    # xft0[p, c, b] = Xf0[b, 128c + p]
    xff0 = np.ascontiguousarray(
        Xf0.T.reshape(32, 128, BLOC).transpose(1, 0, 2)).astype(np.float32)
    xft0 = xff0.astype(X_NP)
    return dict(u64=u64, ut16=ut16, v64=v64, vt16q=vt16q,
                xft0=xft0, xff0=xff0)


def assemble_outputs(results):
    """results: list of 8 per-core dicts -> full outputs."""
    preds, clss, transfs = [], [], []
    for r in results:
        preds.append(r["o_pred"])
        clss.append(r["o_cls"])
        ot = r["o_transf"]  # (9, 128, 32, 32) [l, p, c, b]
        transfs.append(np.ascontiguousarray(
            ot.transpose(3, 2, 1, 0)).reshape(BLOC, DD, L + 1))
    X_predicted = np.concatenate(preds, axis=0).astype(np.float32)
    X_classified = np.concatenate(clss, axis=0).astype(np.float32)
    X_transformed = np.concatenate(transfs, axis=0).astype(np.float32)
    return X_predicted, X_classified, X_transformed


def run(X, W0, W, b, Wc, bc, **run_kwargs):
    if "nc" not in _CACHE:
        _CACHE["nc"] = build_nc()
    nc = _CACHE["nc"]
    shared = host_prep_shared(np.asarray(W0, np.float32),
                              np.asarray(W, np.float32),
                              np.asarray(b, np.float32),
                              np.asarray(Wc, np.float32),
                              np.asarray(bc, np.float32))
    X = np.asarray(X, np.float32)
    in_maps = []
    for c in range(NCORES):
        m = dict(shared)
        m.update(host_prep_core(X[c * BLOC:(c + 1) * BLOC]))
        in_maps.append(m)
    res = run_bass_kernel_spmd(nc, in_maps, core_ids=list(range(NCORES)),
                               **run_kwargs)
    return assemble_outputs(res.results), res


def kernel(X, W0, W, b, Wc, bc):
    outs, _ = run(X, W0, W, b, Wc, bc)
    return outs


# revision 11
# speedup vs baseline: 1.7325x; 1.1778x over previous
"""Trainium2 Bass kernel for nn_DynResNet (B=256, DIM=64, K=16, L=8).

Strategy (validated numerically against the jax reference in fp64/fp32):
- Pure data parallel: 32 samples per core x 8 cores; 9 shared 4096x4096
  weights streamed from HBM as fp8e4m3 (weight rounding only affects the
  output through H=1e-3-damped updates; measured end-to-end error ~5e-5).
- The Cayley u/v updates change u and v by ~1e-7 relative (below fp32 ulp
  of u); dropping them is within ~2e-6 of the fp32 reference. Only the s
  update (s += H * u^T relu(lin) v) and Xf = u s v^T recompute remain.
- Big matmul: stationary = Xf^T k-chunks (128, 32) bf16 in 4 column-tiled
  array groups, reused across all 8 output-chunk PSUM banks via
  ldweights=False follow-on matmuls; moving = W^T tiles (128, 512) fp8.
  Partition-group partial sums are reduced AND transposed in one PE pass
  against a stacked identity, yielding dY in the (j, i, b) layout the
  per-sample small matmuls consume.
"""

import numpy as np
import ml_dtypes

import concourse.bass as bass
import concourse.tile as tile
from concourse import bacc, mybir
from concourse.bass_utils import run_bass_kernel_spmd

DIM, KR, L, DD, B, NCORES = 64, 16, 8, 4096, 256, 8
BLOC = B // NCORES  # 32
H = 1e-3
F32 = mybir.dt.float32
W_DT = mybir.dt.float8e4
X_DT = mybir.dt.bfloat16
W_NP = ml_dtypes.float8_e4m3
X_NP = ml_dtypes.bfloat16

_CACHE = {}
MT_ROW_TILED = True


def build_nc():
    nc = bacc.Bacc("TRN2", target_bir_lowering=False, debug=False,
                   num_devices=NCORES)

    def inp(name, shape, dt):
        return nc.dram_tensor(name, shape, dt, kind="ExternalInput").ap()

    # wq[l, r, p, j, c, n] = W_l^T[(4r+j)*128 + p, c*512 + n]
    wq = inp("wq", (L + 1, 8, 128, 4, 8, 512), W_DT)
    u64 = inp("u64", (64, BLOC, KR), X_DT)
    ut16 = inp("ut16", (KR, BLOC, 64), X_DT)
    v64 = inp("v64", (64, BLOC, KR), X_DT)
    vt16q = inp("vt16q", (16, BLOC, 64), X_DT)
    xft0 = inp("xft0", (128, 32, BLOC), X_DT)
    xff0 = inp("xff0", (128, 32, BLOC), F32)
    bt = inp("bt", (64, L + 1, 64), F32)
    e4 = inp("e4", (128, BLOC), X_DT)
    wct = inp("wct", (128, 32, 10), F32)
    bc1 = inp("bc1", (1, 10), F32)
    ones1 = inp("ones1", (1, BLOC), F32)

    o_transf = nc.dram_tensor("o_transf", (L + 1, 128, 32, BLOC), F32,
                              kind="ExternalOutput").ap()
    o_cls = nc.dram_tensor("o_cls", (BLOC, 10), F32,
                           kind="ExternalOutput").ap()
    o_pred = nc.dram_tensor("o_pred", (BLOC, 10), F32,
                            kind="ExternalOutput").ap()

    add, mult, mx_op = (mybir.AluOpType.add, mybir.AluOpType.mult,
                        mybir.AluOpType.max)

    with tile.TileContext(nc) as tc:
        with (
            tc.tile_pool(name="consts", bufs=1) as consts,
            tc.tile_pool(name="wpool", bufs=6) as wpool,
            tc.tile_pool(name="xft", bufs=2) as xftp,
            tc.tile_pool(name="xff", bufs=2) as xffp,
            tc.tile_pool(name="dyt", bufs=2) as dytp,
            tc.tile_pool(name="lp", bufs=3) as lpp,
            tc.tile_pool(name="work", bufs=2) as work,
            tc.tile_pool(name="psum", bufs=8, space="PSUM") as psum,
        ):
            def cload(ap_in, shape, dt):
                t = consts.tile(shape, dt, tag=ap_in.tensor.name)
                nc.sync.dma_start(t[:], ap_in)
                return t

            u64sb = cload(u64, [64, BLOC, KR], X_DT)
            ut16sb = cload(ut16, [KR, BLOC, 64], X_DT)
            v64sb = cload(v64, [64, BLOC, KR], X_DT)
            vt16qsb = cload(vt16q, [16, BLOC, 64], X_DT)
            btsb = cload(bt, [64, L + 1, 64], F32)
            e4sb = cload(e4, [128, BLOC], X_DT)
            wctsb = cload(wct, [128, 32, 10], F32)
            bc1sb = cload(bc1, [1, 10], F32)
            ones1sb = cload(ones1, [1, BLOC], F32)

            xft_bf = xftp.tile([128, 32, BLOC], X_DT, tag="xft")
            nc.sync.dma_start(xft_bf[:], xft0)
            xff_cur = xffp.tile([128, 32, BLOC], F32, tag="xff")
            nc.sync.dma_start(xff_cur[:], xff0)

            for l in range(L + 1):
                # ---- big matmul: all 8 n-chunk banks, k-chunk-outer ----
                pa = [psum.tile([128, 512], F32, tag="bank",
                                name=f"pa{l}_{i}")
                      for i in range(8)]
                for r in range(8):
                    wt = wpool.tile([128, 4, 8, 512], W_DT, tag="wt")
                    nc.sync.dma_start(wt[:], wq[l, r])
                    for cn in range(8):
                        for j in range(4):
                            kc = 4 * r + j
                            mm = nc.tensor.matmul(
                                pa[cn][32 * j:32 * j + 32, :],
                                xft_bf[:, kc, :],
                                wt[:, j, cn, :],
                                start=(r == 0), stop=(r == 7),
                                tile_position=(0, 32 * j),
                                skip_group_check=True,
                            )
                            if cn > 0:
                                mm.ldweights = False

                # ---- reduce partition groups + transpose + bias + relu ----
                dyt = dytp.tile([64, 64, BLOC], X_DT, tag="dyt")
                for cn in range(8):
                    lp = lpp.tile([128, 512], X_DT, tag="lp")
                    nc.vector.tensor_copy(lp[:], pa[cn][:])
                    prd = psum.tile([64, 8, BLOC], F32, tag="bank")
                    for m in range(8):
                        nc.tensor.matmul(prd[:, m, :],
                                         lp[:, 64 * m:64 * m + 64],
                                         e4sb[:], start=True, stop=True)
                    dsl = dyt[:, 8 * cn:8 * cn + 8, :]
                    bias_bc = btsb[:, l, 8 * cn:8 * cn + 8][:, :, None] \
                        .broadcast_to((64, 8, BLOC))
                    nc.vector.tensor_add(dsl, prd[:], bias_bc)
                    nc.vector.tensor_scalar_max(dsl, dsl, 0.0)

                # ---- z_u = dY v ; dS = u^T z_u ; s += H dS ----
                pz = psum.tile([64, BLOC, KR], F32, tag="bank")
                for bb in range(BLOC):
                    nc.tensor.matmul(pz[:, bb, :], dyt[:, :, bb],
                                     v64sb[:, bb, :], start=True, stop=True)
                zu = work.tile([64, BLOC, KR], X_DT, tag="zu")
                nc.vector.tensor_copy(zu[:], pz[:])
                pds = psum.tile([KR, BLOC, KR], F32, tag="bank")
                for bb in range(BLOC):
                    nc.tensor.matmul(pds[:, bb, :], u64sb[:, bb, :],
                                     zu[:, bb, :], start=True, stop=True)
                dsb = work.tile([KR, BLOC, KR], X_DT, tag="dsb")
                nc.vector.tensor_scalar_mul(dsb[:], pds[:], float(H))

                # ---- r1 = (H dS)^T u^T per sample ----
                r1 = work.tile([KR, BLOC, 64], X_DT, tag="r1")
                for grp in range(4):
                    p1 = psum.tile([KR, 8, 64], F32, tag="bank")
                    for bb in range(8):
                        bs = grp * 8 + bb
                        nc.tensor.matmul(p1[:, bb, :], dsb[:, bs, :],
                                         ut16sb[:, bs, :], start=True,
                                         stop=True)
                    nc.scalar.activation(r1[:, grp * 8:grp * 8 + 8, :],
                                         p1[:],
                                         mybir.ActivationFunctionType.Copy)

                # ---- dM^T = v r1 ; Xf += dM (both halves), batched ----
                xft_new = xftp.tile([128, 32, BLOC], X_DT, tag="xft")
                xff_new = xffp.tile([128, 32, BLOC], F32, tag="xff")
                for grp in range(4):
                    pm = psum.tile([128, 8, 64], F32, tag="bank")
                    for bb in range(8):
                        bs = grp * 8 + bb
                        vq = vt16qsb[:, bs, :]
                        rq = r1[:, bs, :]
                        nc.tensor.matmul(pm[0:64, bb, :], vq, rq,
                                         start=True, stop=True)
                        nc.tensor.matmul(pm[64:128, bb, :], vq, rq,
                                         start=True, stop=True,
                                         tile_position=(0, 64))
                    bsl = slice(8 * grp, 8 * grp + 8)
                    pm_even = pm[0:64, :, 0:64:2].transpose([0, 2, 1])
                    pm_odd = pm[64:128, :, 1:64:2].transpose([0, 2, 1])
                    nc.vector.tensor_add(xff_new[0:64, :, bsl],
                                         xff_cur[0:64, :, bsl], pm_even)
                    nc.vector.tensor_add(xff_new[64:128, :, bsl],
                                         xff_cur[64:128, :, bsl], pm_odd)
                    nc.vector.tensor_copy(xft_new[0:64, :, bsl],
                                          xff_new[0:64, :, bsl])
                    nc.vector.tensor_copy(xft_new[64:128, :, bsl],
                                          xff_new[64:128, :, bsl])
                nc.sync.dma_start(o_transf[l], xff_new[:])
                xft_bf = xft_new
                xff_cur = xff_new

            # ---- classification head + softmax ----
            pc = psum.tile([BLOC, 10], F32, tag="bank")
            for c in range(32):
                nc.tensor.matmul(pc[:], xff_cur[:, c, :], wctsb[:, c, :],
                                 start=(c == 0), stop=False)
            nc.tensor.matmul(pc[:], ones1sb[:], bc1sb[:], start=False,
                             stop=True)
            cls = work.tile([BLOC, 10], F32, tag="cls")
            nc.vector.tensor_copy(cls[:], pc[:])
            nc.sync.dma_start(o_cls, cls[:])

            mxt = work.tile([BLOC, 1], F32, tag="mx")
            nc.vector.tensor_reduce(mxt[:], cls[:], mybir.AxisListType.X,
                                    mx_op)
            sh = work.tile([BLOC, 10], F32, tag="sh")
            nc.vector.tensor_scalar_sub(sh[:], cls[:], mxt[:])
            ex = work.tile([BLOC, 10], F32, tag="ex")
            nc.scalar.activation(ex[:], sh[:],
                                 mybir.ActivationFunctionType.Exp)
            sm = work.tile([BLOC, 1], F32, tag="sum")
            nc.vector.tensor_reduce(sm[:], ex[:], mybir.AxisListType.X, add)
            nc.vector.reciprocal(sm[:], sm[:])
            prd_t = work.tile([BLOC, 10], F32, tag="pd")
            nc.vector.tensor_scalar_mul(prd_t[:], ex[:], sm[:])
            nc.sync.dma_start(o_pred, prd_t[:])

    nc.compile()
    return nc


def host_prep_shared(W0, W, b, Wc, bc):
    """Per-layer shared tensors (identical on every core)."""
    wq = np.empty((L + 1, 8, 128, 4, 8, 512), dtype=W_NP)
    for l in range(L + 1):
        Wm = W0 if l == 0 else W[l - 1]
        WT = np.ascontiguousarray(Wm.T).astype(W_NP)  # (4096 k, 4096 n)
        # (8r, 4j, 128p, 8c, 512n) -> (r, p, j, c, n)
        wq[l] = WT.reshape(8, 4, 128, 8, 512).transpose(0, 2, 1, 3, 4)
    bt = np.zeros((64, L + 1, 64), dtype=np.float32)
    for l in range(1, L + 1):
        bt[:, l, :] = b[l - 1].reshape(64, 64).T  # [j, i]
    e4 = np.tile(np.eye(BLOC), (4, 1)).astype(X_NP)  # (128, 32)
    wct = np.ascontiguousarray(
        Wc.T.reshape(32, 128, 10).transpose(1, 0, 2)).astype(np.float32)
    bc1 = bc.reshape(1, 10).astype(np.float32)
    ones1 = np.ones((1, BLOC), dtype=np.float32)
    return dict(wq=wq, bt=bt, e4=e4, wct=wct, bc1=bc1, ones1=ones1)


def host_prep_core(Xc):
    """Per-core tensors from this core's 32-sample X slice (32, 3, 1024)."""
    u = Xc[:, 0].reshape(BLOC, 64, 16)
    s = Xc[:, 1].reshape(BLOC, 64, 16)[:, :16, :16]
    vh = Xc[:, 2].reshape(BLOC, 16, 64)
    u64 = np.ascontiguousarray(u.transpose(1, 0, 2)).astype(X_NP)
    ut16 = np.ascontiguousarray(u.transpose(2, 0, 1)).astype(X_NP)
    v64 = np.ascontiguousarray(vh.transpose(2, 0, 1)).astype(X_NP)
    vt16q = np.ascontiguousarray(vh.transpose(1, 0, 2)).astype(X_NP)
    s16 = np.ascontiguousarray(s.transpose(1, 0, 2), dtype=np.float32)
    Xf0 = np.einsum('bik,bkl,blj->bij', u, s, vh,
                    optimize=True).reshape(BLOC, DD)
    # xft0[p, c, b] = Xf0[b, 128c + p]
    xff0 = np.ascontiguousarray(
        Xf0.T.reshape(32, 128, BLOC).transpose(1, 0, 2)).astype(np.float32)
    xft0 = xff0.astype(X_NP)
    return dict(u64=u64, ut16=ut16, v64=v64, vt16q=vt16q,
                xft0=xft0, xff0=xff0)


def assemble_outputs(results):
    """results: list of 8 per-core dicts -> full outputs."""
    preds, clss, transfs = [], [], []
    for r in results:
        preds.append(r["o_pred"])
        clss.append(r["o_cls"])
        ot = r["o_transf"]  # (9, 128, 32, 32) [l, p, c, b]
        transfs.append(np.ascontiguousarray(
            ot.transpose(3, 2, 1, 0)).reshape(BLOC, DD, L + 1))
    X_predicted = np.concatenate(preds, axis=0).astype(np.float32)
    X_classified = np.concatenate(clss, axis=0).astype(np.float32)
    X_transformed = np.concatenate(transfs, axis=0).astype(np.float32)
    return X_predicted, X_classified, X_transformed


def run(X, W0, W, b, Wc, bc, **run_kwargs):
    if "nc" not in _CACHE:
        _CACHE["nc"] = build_nc()
    nc = _CACHE["nc"]
    shared = host_prep_shared(np.asarray(W0, np.float32),
                              np.asarray(W, np.float32),
                              np.asarray(b, np.float32),
                              np.asarray(Wc, np.float32),
                              np.asarray(bc, np.float32))
    X = np.asarray(X, np.float32)
    in_maps = []
    for c in range(NCORES):
        m = dict(shared)
        m.update(host_prep_core(X[c * BLOC:(c + 1) * BLOC]))
        in_maps.append(m)
    res = run_bass_kernel_spmd(nc, in_maps, core_ids=list(range(NCORES)),
                               **run_kwargs)
    return assemble_outputs(res.results), res


def kernel(X, W0, W, b, Wc, bc):
    outs, _ = run(X, W0, W, b, Wc, bc)
    return outs


# revision 12
# speedup vs baseline: 1.7949x; 1.0361x over previous
"""Trainium2 Bass kernel for nn_DynResNet (B=256, DIM=64, K=16, L=8).

Strategy (validated numerically against the jax reference in fp64/fp32):
- Pure data parallel: 32 samples per core x 8 cores; 9 shared 4096x4096
  weights streamed from HBM as fp8e4m3 (weight rounding only affects the
  output through H=1e-3-damped updates; measured end-to-end error ~5e-5).
- The Cayley u/v updates change u and v by ~1e-7 relative (below fp32 ulp
  of u); dropping them is within ~2e-6 of the fp32 reference. Only the s
  update (s += H * u^T relu(lin) v) and Xf = u s v^T recompute remain.
- Big matmul: stationary = Xf^T k-chunks (128, 32) bf16 in 4 column-tiled
  array groups, reused across all 8 output-chunk PSUM banks via
  ldweights=False follow-on matmuls; moving = W^T tiles (128, 512) fp8.
  Partition-group partial sums are reduced AND transposed in one PE pass
  against a stacked identity, yielding dY in the (j, i, b) layout the
  per-sample small matmuls consume.
"""

import numpy as np
import ml_dtypes

import concourse.bass as bass
import concourse.tile as tile
from concourse import bacc, mybir
from concourse.bass_utils import run_bass_kernel_spmd

DIM, KR, L, DD, B, NCORES = 64, 16, 8, 4096, 256, 8
BLOC = B // NCORES  # 32
H = 1e-3
F32 = mybir.dt.float32
W_DT = mybir.dt.float8e4
X_DT = mybir.dt.bfloat16
W_NP = ml_dtypes.float8_e4m3
X_NP = ml_dtypes.bfloat16

_CACHE = {}
MT_ROW_TILED = True


def build_nc():
    nc = bacc.Bacc("TRN2", target_bir_lowering=False, debug=False,
                   num_devices=NCORES)

    def inp(name, shape, dt):
        return nc.dram_tensor(name, shape, dt, kind="ExternalInput").ap()

    # wq[l, r, p, j, c, n] = W_l^T[(4r+j)*128 + p, c*512 + n]
    wq = inp("wq", (L + 1, 8, 128, 4, 8, 512), W_DT)
    u64 = inp("u64", (64, BLOC, KR), X_DT)
    ut16 = inp("ut16", (KR, BLOC, 64), X_DT)
    v64 = inp("v64", (64, BLOC, KR), X_DT)
    vt16q = inp("vt16q", (16, BLOC, 64), X_DT)
    xft0 = inp("xft0", (128, 32, BLOC), X_DT)
    xff0 = inp("xff0", (128, 32, BLOC), F32)
    bt = inp("bt", (64, L + 1, 64), F32)
    e4 = inp("e4", (128, BLOC), X_DT)
    wct = inp("wct", (128, 32, 10), F32)
    bc1 = inp("bc1", (1, 10), F32)
    ones1 = inp("ones1", (1, BLOC), F32)

    o_transf = nc.dram_tensor("o_transf", (L + 1, 128, 32, BLOC), F32,
                              kind="ExternalOutput").ap()
    o_cls = nc.dram_tensor("o_cls", (BLOC, 10), F32,
                           kind="ExternalOutput").ap()
    o_pred = nc.dram_tensor("o_pred", (BLOC, 10), F32,
                            kind="ExternalOutput").ap()

    add, mult, mx_op = (mybir.AluOpType.add, mybir.AluOpType.mult,
                        mybir.AluOpType.max)

    with tile.TileContext(nc) as tc:
        with (
            tc.tile_pool(name="consts", bufs=1) as consts,
            tc.tile_pool(name="wpool", bufs=8) as wpool,
            tc.tile_pool(name="xft", bufs=2) as xftp,
            tc.tile_pool(name="xff", bufs=2) as xffp,
            tc.tile_pool(name="dyt", bufs=2) as dytp,
            tc.tile_pool(name="lp", bufs=3) as lpp,
            tc.tile_pool(name="work", bufs=2) as work,
            tc.tile_pool(name="psum", bufs=8, space="PSUM") as psum,
        ):
            def cload(ap_in, shape, dt):
                t = consts.tile(shape, dt, tag=ap_in.tensor.name)
                nc.sync.dma_start(t[:], ap_in)
                return t

            u64sb = cload(u64, [64, BLOC, KR], X_DT)
            ut16sb = cload(ut16, [KR, BLOC, 64], X_DT)
            v64sb = cload(v64, [64, BLOC, KR], X_DT)
            vt16qsb = cload(vt16q, [16, BLOC, 64], X_DT)
            btsb = cload(bt, [64, L + 1, 64], F32)
            e4sb = cload(e4, [128, BLOC], X_DT)
            wctsb = cload(wct, [128, 32, 10], F32)
            bc1sb = cload(bc1, [1, 10], F32)
            ones1sb = cload(ones1, [1, BLOC], F32)

            xft_bf = xftp.tile([128, 32, BLOC], X_DT, tag="xft")
            nc.sync.dma_start(xft_bf[:], xft0)
            xff_cur = xffp.tile([128, 32, BLOC], F32, tag="xff")
            nc.sync.dma_start(xff_cur[:], xff0)

            for l in range(L + 1):
                # ---- big matmul: all 8 n-chunk banks, k-chunk-outer ----
                pa = [psum.tile([128, 512], F32, tag="bank",
                                name=f"pa{l}_{i}")
                      for i in range(8)]
                for r in range(8):
                    wt = wpool.tile([128, 4, 8, 512], W_DT, tag="wt")
                    nc.sync.dma_start(wt[:], wq[l, r])
                    for cn in range(8):
                        for j in range(4):
                            kc = 4 * r + j
                            mm = nc.tensor.matmul(
                                pa[cn][32 * j:32 * j + 32, :],
                                xft_bf[:, kc, :],
                                wt[:, j, cn, :],
                                start=(r == 0), stop=(r == 7),
                                tile_position=(0, 32 * j),
                                skip_group_check=True,
                            )
                            if cn > 0:
                                mm.ldweights = False

                # ---- reduce partition groups + transpose + bias + relu ----
                dyt = dytp.tile([64, 64, BLOC], X_DT, tag="dyt")
                for cn in range(8):
                    lp = lpp.tile([128, 512], X_DT, tag="lp")
                    nc.vector.tensor_copy(lp[:], pa[cn][:])
                    prd = psum.tile([64, 8, BLOC], F32, tag="bank")
                    for m in range(8):
                        nc.tensor.matmul(prd[:, m, :],
                                         lp[:, 64 * m:64 * m + 64],
                                         e4sb[:], start=True, stop=True)
                    dsl = dyt[:, 8 * cn:8 * cn + 8, :]
                    bias_bc = btsb[:, l, 8 * cn:8 * cn + 8][:, :, None] \
                        .broadcast_to((64, 8, BLOC))
                    nc.vector.tensor_add(dsl, prd[:], bias_bc)
                    nc.vector.tensor_scalar_max(dsl, dsl, 0.0)

                # ---- z_u = dY v ; dS = u^T z_u ; s += H dS ----
                pz = psum.tile([64, BLOC, KR], F32, tag="bank")
                for bb in range(BLOC):
                    nc.tensor.matmul(pz[:, bb, :], dyt[:, :, bb],
                                     v64sb[:, bb, :], start=True, stop=True)
                zu = work.tile([64, BLOC, KR], X_DT, tag="zu")
                nc.vector.tensor_copy(zu[:], pz[:])
                pds = psum.tile([KR, BLOC, KR], F32, tag="bank")
                for bb in range(BLOC):
                    nc.tensor.matmul(pds[:, bb, :], u64sb[:, bb, :],
                                     zu[:, bb, :], start=True, stop=True)
                dsb = work.tile([KR, BLOC, KR], X_DT, tag="dsb")
                nc.vector.tensor_scalar_mul(dsb[:], pds[:], float(H))

                # ---- r1 = (H dS)^T u^T per sample ----
                r1 = work.tile([KR, BLOC, 64], X_DT, tag="r1")
                for grp in range(4):
                    p1 = psum.tile([KR, 8, 64], F32, tag="bank")
                    for bb in range(8):
                        bs = grp * 8 + bb
                        nc.tensor.matmul(p1[:, bb, :], dsb[:, bs, :],
                                         ut16sb[:, bs, :], start=True,
                                         stop=True)
                    nc.scalar.activation(r1[:, grp * 8:grp * 8 + 8, :],
                                         p1[:],
                                         mybir.ActivationFunctionType.Copy)

                # ---- dM^T = v r1 ; Xf += dM (both halves), batched ----
                xft_new = xftp.tile([128, 32, BLOC], X_DT, tag="xft")
                xff_new = xffp.tile([128, 32, BLOC], F32, tag="xff")
                for grp in range(4):
                    pm = psum.tile([128, 8, 64], F32, tag="bank")
                    for bb in range(8):
                        bs = grp * 8 + bb
                        vq = vt16qsb[:, bs, :]
                        rq = r1[:, bs, :]
                        nc.tensor.matmul(pm[0:64, bb, :], vq, rq,
                                         start=True, stop=True)
                        nc.tensor.matmul(pm[64:128, bb, :], vq, rq,
                                         start=True, stop=True,
                                         tile_position=(0, 64))
                    bsl = slice(8 * grp, 8 * grp + 8)
                    pm_even = pm[0:64, :, 0:64:2].transpose([0, 2, 1])
                    pm_odd = pm[64:128, :, 1:64:2].transpose([0, 2, 1])
                    nc.vector.tensor_add(xff_new[0:64, :, bsl],
                                         xff_cur[0:64, :, bsl], pm_even)
                    nc.vector.tensor_add(xff_new[64:128, :, bsl],
                                         xff_cur[64:128, :, bsl], pm_odd)
                    nc.vector.tensor_copy(xft_new[0:64, :, bsl],
                                          xff_new[0:64, :, bsl])
                    nc.vector.tensor_copy(xft_new[64:128, :, bsl],
                                          xff_new[64:128, :, bsl])
                nc.sync.dma_start(o_transf[l], xff_new[:])
                xft_bf = xft_new
                xff_cur = xff_new

            # ---- classification head + softmax ----
            pc = psum.tile([BLOC, 10], F32, tag="bank")
            for c in range(32):
                nc.tensor.matmul(pc[:], xff_cur[:, c, :], wctsb[:, c, :],
                                 start=(c == 0), stop=False)
            nc.tensor.matmul(pc[:], ones1sb[:], bc1sb[:], start=False,
                             stop=True)
            cls = work.tile([BLOC, 10], F32, tag="cls")
            nc.vector.tensor_copy(cls[:], pc[:])
            nc.sync.dma_start(o_cls, cls[:])

            mxt = work.tile([BLOC, 1], F32, tag="mx")
            nc.vector.tensor_reduce(mxt[:], cls[:], mybir.AxisListType.X,
                                    mx_op)
            sh = work.tile([BLOC, 10], F32, tag="sh")
            nc.vector.tensor_scalar_sub(sh[:], cls[:], mxt[:])
            ex = work.tile([BLOC, 10], F32, tag="ex")
            nc.scalar.activation(ex[:], sh[:],
                                 mybir.ActivationFunctionType.Exp)
            sm = work.tile([BLOC, 1], F32, tag="sum")
            nc.vector.tensor_reduce(sm[:], ex[:], mybir.AxisListType.X, add)
            nc.vector.reciprocal(sm[:], sm[:])
            prd_t = work.tile([BLOC, 10], F32, tag="pd")
            nc.vector.tensor_scalar_mul(prd_t[:], ex[:], sm[:])
            nc.sync.dma_start(o_pred, prd_t[:])

    nc.compile()
    return nc


def host_prep_shared(W0, W, b, Wc, bc):
    """Per-layer shared tensors (identical on every core)."""
    wq = np.empty((L + 1, 8, 128, 4, 8, 512), dtype=W_NP)
    for l in range(L + 1):
        Wm = W0 if l == 0 else W[l - 1]
        WT = np.ascontiguousarray(Wm.T).astype(W_NP)  # (4096 k, 4096 n)
        # (8r, 4j, 128p, 8c, 512n) -> (r, p, j, c, n)
        wq[l] = WT.reshape(8, 4, 128, 8, 512).transpose(0, 2, 1, 3, 4)
    bt = np.zeros((64, L + 1, 64), dtype=np.float32)
    for l in range(1, L + 1):
        bt[:, l, :] = b[l - 1].reshape(64, 64).T  # [j, i]
    e4 = np.tile(np.eye(BLOC), (4, 1)).astype(X_NP)  # (128, 32)
    wct = np.ascontiguousarray(
        Wc.T.reshape(32, 128, 10).transpose(1, 0, 2)).astype(np.float32)
    bc1 = bc.reshape(1, 10).astype(np.float32)
    ones1 = np.ones((1, BLOC), dtype=np.float32)
    return dict(wq=wq, bt=bt, e4=e4, wct=wct, bc1=bc1, ones1=ones1)


def host_prep_core(Xc):
    """Per-core tensors from this core's 32-sample X slice (32, 3, 1024)."""
    u = Xc[:, 0].reshape(BLOC, 64, 16)
    s = Xc[:, 1].reshape(BLOC, 64, 16)[:, :16, :16]
    vh = Xc[:, 2].reshape(BLOC, 16, 64)
    u64 = np.ascontiguousarray(u.transpose(1, 0, 2)).astype(X_NP)
    ut16 = np.ascontiguousarray(u.transpose(2, 0, 1)).astype(X_NP)
    v64 = np.ascontiguousarray(vh.transpose(2, 0, 1)).astype(X_NP)
    vt16q = np.ascontiguousarray(vh.transpose(1, 0, 2)).astype(X_NP)
    s16 = np.ascontiguousarray(s.transpose(1, 0, 2), dtype=np.float32)
    Xf0 = np.einsum('bik,bkl,blj->bij', u, s, vh,
                    optimize=True).reshape(BLOC, DD)
    # xft0[p, c, b] = Xf0[b, 128c + p]
    xff0 = np.ascontiguousarray(
        Xf0.T.reshape(32, 128, BLOC).transpose(1, 0, 2)).astype(np.float32)
    xft0 = xff0.astype(X_NP)
    return dict(u64=u64, ut16=ut16, v64=v64, vt16q=vt16q,
                xft0=xft0, xff0=xff0)


def assemble_outputs(results):
    """results: list of 8 per-core dicts -> full outputs."""
    preds, clss, transfs = [], [], []
    for r in results:
        preds.append(r["o_pred"])
        clss.append(r["o_cls"])
        ot = r["o_transf"]  # (9, 128, 32, 32) [l, p, c, b]
        transfs.append(np.ascontiguousarray(
            ot.transpose(3, 2, 1, 0)).reshape(BLOC, DD, L + 1))
    X_predicted = np.concatenate(preds, axis=0).astype(np.float32)
    X_classified = np.concatenate(clss, axis=0).astype(np.float32)
    X_transformed = np.concatenate(transfs, axis=0).astype(np.float32)
    return X_predicted, X_classified, X_transformed


def run(X, W0, W, b, Wc, bc, **run_kwargs):
    if "nc" not in _CACHE:
        _CACHE["nc"] = build_nc()
    nc = _CACHE["nc"]
    shared = host_prep_shared(np.asarray(W0, np.float32),
                              np.asarray(W, np.float32),
                              np.asarray(b, np.float32),
                              np.asarray(Wc, np.float32),
                              np.asarray(bc, np.float32))
    X = np.asarray(X, np.float32)
    in_maps = []
    for c in range(NCORES):
        m = dict(shared)
        m.update(host_prep_core(X[c * BLOC:(c + 1) * BLOC]))
        in_maps.append(m)
    res = run_bass_kernel_spmd(nc, in_maps, core_ids=list(range(NCORES)),
                               **run_kwargs)
    return assemble_outputs(res.results), res


def kernel(X, W0, W, b, Wc, bc):
    outs, _ = run(X, W0, W, b, Wc, bc)
    return outs


# revision 14
# speedup vs baseline: 1.9179x; 1.0685x over previous
"""Trainium2 Bass kernel for nn_DynResNet (B=256, DIM=64, K=16, L=8).

Strategy (validated numerically against the jax reference in fp64/fp32):
- Pure data parallel: 32 samples per core x 8 cores; 9 shared 4096x4096
  weights streamed from HBM as fp8e4m3 (weight rounding only affects the
  output through H=1e-3-damped updates; measured end-to-end error ~5e-5).
- The Cayley u/v updates change u and v by ~1e-7 relative (below fp32 ulp
  of u); dropping them is within ~2e-6 of the fp32 reference. Only the s
  update (s += H * u^T relu(lin) v) and Xf = u s v^T recompute remain.
- Big matmul: stationary = Xf^T k-chunks (128, 32) bf16 in 4 column-tiled
  array groups, reused across all 8 output-chunk PSUM banks via
  ldweights=False follow-on matmuls; moving = W^T tiles (128, 512) fp8.
  Partition-group partial sums are reduced AND transposed in one PE pass
  against a stacked identity, yielding dY in the (j, i, b) layout the
  per-sample small matmuls consume.
"""

import numpy as np
import ml_dtypes

import concourse.bass as bass
import concourse.tile as tile
from concourse import bacc, mybir
from concourse.bass_utils import run_bass_kernel_spmd

DIM, KR, L, DD, B, NCORES = 64, 16, 8, 4096, 256, 8
BLOC = B // NCORES  # 32
H = 1e-3
F32 = mybir.dt.float32
W_DT = mybir.dt.float8e4
X_DT = mybir.dt.bfloat16
W_NP = ml_dtypes.float8_e4m3
X_NP = ml_dtypes.bfloat16

_CACHE = {}
MT_ROW_TILED = True


def build_nc():
    nc = bacc.Bacc("TRN2", target_bir_lowering=False, debug=False,
                   num_devices=NCORES)

    def inp(name, shape, dt):
        return nc.dram_tensor(name, shape, dt, kind="ExternalInput").ap()

    # wq[l, r, p, j, c, n] = W_l^T[(4r+j)*128 + p, c*512 + n]
    wq = inp("wq", (L + 1, 8, 128, 4, 8, 512), W_DT)
    u64 = inp("u64", (64, BLOC, KR), X_DT)
    ut16 = inp("ut16", (KR, BLOC, 64), X_DT)
    v64 = inp("v64", (64, BLOC, KR), X_DT)
    vt16q = inp("vt16q", (16, BLOC, 64), X_DT)
    xft0 = inp("xft0", (128, 32, BLOC), X_DT)
    xff0 = inp("xff0", (128, 32, BLOC), F32)
    bt = inp("bt", (64, L + 1, 64), F32)
    e4 = inp("e4", (128, BLOC), X_DT)
    wct = inp("wct", (128, 32, 10), F32)
    bc1 = inp("bc1", (1, 10), F32)
    ones1 = inp("ones1", (1, BLOC), F32)

    o_transf = nc.dram_tensor("o_transf", (L + 1, 128, 32, BLOC), F32,
                              kind="ExternalOutput").ap()
    o_cls = nc.dram_tensor("o_cls", (BLOC, 10), F32,
                           kind="ExternalOutput").ap()
    o_pred = nc.dram_tensor("o_pred", (BLOC, 10), F32,
                            kind="ExternalOutput").ap()

    add, mult, mx_op = (mybir.AluOpType.add, mybir.AluOpType.mult,
                        mybir.AluOpType.max)

    with tile.TileContext(nc) as tc:
        with (
            tc.tile_pool(name="consts", bufs=1) as consts,
            tc.tile_pool(name="wpool", bufs=9) as wpool,
            tc.tile_pool(name="xft", bufs=2) as xftp,
            tc.tile_pool(name="xff", bufs=2) as xffp,
            tc.tile_pool(name="dyt", bufs=2) as dytp,
            tc.tile_pool(name="lp", bufs=8) as lpp,
            tc.tile_pool(name="work", bufs=2) as work,
            tc.tile_pool(name="psum", bufs=8, space="PSUM") as psum,
        ):
            def cload(ap_in, shape, dt):
                t = consts.tile(shape, dt, tag=ap_in.tensor.name)
                nc.sync.dma_start(t[:], ap_in)
                return t

            u64sb = cload(u64, [64, BLOC, KR], X_DT)
            ut16sb = cload(ut16, [KR, BLOC, 64], X_DT)
            v64sb = cload(v64, [64, BLOC, KR], X_DT)
            vt16qsb = cload(vt16q, [16, BLOC, 64], X_DT)
            btsb = cload(bt, [64, L + 1, 64], F32)
            e4sb = cload(e4, [128, BLOC], X_DT)
            wctsb = cload(wct, [128, 32, 10], F32)
            bc1sb = cload(bc1, [1, 10], F32)
            ones1sb = cload(ones1, [1, BLOC], F32)

            xft_bf = xftp.tile([128, 32, BLOC], X_DT, tag="xft")
            nc.sync.dma_start(xft_bf[:], xft0)
            xff_cur = xffp.tile([128, 32, BLOC], F32, tag="xff")
            nc.sync.dma_start(xff_cur[:], xff0)

            for l in range(L + 1):
                # ---- big matmul: all 8 n-chunk banks, k-chunk-outer ----
                pa = [psum.tile([128, 512], F32, tag="bank",
                                name=f"pa{l}_{i}")
                      for i in range(8)]
                for r in range(8):
                    wt = wpool.tile([128, 4, 8, 512], W_DT, tag="wt")
                    nc.sync.dma_start(wt[:], wq[l, r])
                    for cn in range(8):
                        for j in range(4):
                            kc = 4 * r + j
                            mm = nc.tensor.matmul(
                                pa[cn][32 * j:32 * j + 32, :],
                                xft_bf[:, kc, :],
                                wt[:, j, cn, :],
                                start=(r == 0), stop=(r == 7),
                                tile_position=(0, 32 * j),
                                skip_group_check=True,
                            )
                            if cn > 0:
                                mm.ldweights = False

                # ---- reduce partition groups + transpose + bias + relu ----
                dyt = dytp.tile([64, 64, BLOC], X_DT, tag="dyt")
                for cn in range(8):
                    lp = lpp.tile([128, 512], X_DT, tag="lp")
                    nc.vector.tensor_copy(lp[:], pa[cn][:])
                    prd = psum.tile([64, 8, BLOC], F32, tag="bank")
                    for m in range(8):
                        nc.tensor.matmul(prd[:, m, :],
                                         lp[:, 64 * m:64 * m + 64],
                                         e4sb[:], start=True, stop=True)
                    dsl = dyt[:, 8 * cn:8 * cn + 8, :]
                    bias_bc = btsb[:, l, 8 * cn:8 * cn + 8][:, :, None] \
                        .broadcast_to((64, 8, BLOC))
                    nc.vector.tensor_add(dsl, prd[:], bias_bc)
                    nc.vector.tensor_scalar_max(dsl, dsl, 0.0)

                # ---- z_u = dY v ; dS = u^T z_u ; s += H dS ----
                pz = psum.tile([64, BLOC, KR], F32, tag="bank")
                for bb in range(BLOC):
                    nc.tensor.matmul(pz[:, bb, :], dyt[:, :, bb],
                                     v64sb[:, bb, :], start=True, stop=True)
                zu = work.tile([64, BLOC, KR], X_DT, tag="zu")
                nc.vector.tensor_copy(zu[:], pz[:])
                pds = psum.tile([KR, BLOC, KR], F32, tag="bank")
                for bb in range(BLOC):
                    nc.tensor.matmul(pds[:, bb, :], u64sb[:, bb, :],
                                     zu[:, bb, :], start=True, stop=True)
                dsb = work.tile([KR, BLOC, KR], X_DT, tag="dsb")
                nc.vector.tensor_scalar_mul(dsb[:], pds[:], float(H))

                # ---- r1 = (H dS)^T u^T per sample ----
                r1 = work.tile([KR, BLOC, 64], X_DT, tag="r1")
                for grp in range(4):
                    p1 = psum.tile([KR, 8, 64], F32, tag="bank")
                    for bb in range(8):
                        bs = grp * 8 + bb
                        nc.tensor.matmul(p1[:, bb, :], dsb[:, bs, :],
                                         ut16sb[:, bs, :], start=True,
                                         stop=True)
                    nc.scalar.activation(r1[:, grp * 8:grp * 8 + 8, :],
                                         p1[:],
                                         mybir.ActivationFunctionType.Copy)

                # ---- dM^T = v r1 ; Xf += dM (both halves), batched ----
                xft_new = xftp.tile([128, 32, BLOC], X_DT, tag="xft")
                xff_new = xffp.tile([128, 32, BLOC], F32, tag="xff")
                for grp in range(4):
                    pm = psum.tile([128, 8, 64], F32, tag="bank")
                    for bb in range(8):
                        bs = grp * 8 + bb
                        vq = vt16qsb[:, bs, :]
                        rq = r1[:, bs, :]
                        nc.tensor.matmul(pm[0:64, bb, :], vq, rq,
                                         start=True, stop=True)
                        nc.tensor.matmul(pm[64:128, bb, :], vq, rq,
                                         start=True, stop=True,
                                         tile_position=(0, 64))
                    bsl = slice(8 * grp, 8 * grp + 8)
                    pm_even = pm[0:64, :, 0:64:2].transpose([0, 2, 1])
                    pm_odd = pm[64:128, :, 1:64:2].transpose([0, 2, 1])
                    nc.vector.tensor_add(xff_new[0:64, :, bsl],
                                         xff_cur[0:64, :, bsl], pm_even)
                    nc.vector.tensor_add(xff_new[64:128, :, bsl],
                                         xff_cur[64:128, :, bsl], pm_odd)
                    nc.vector.tensor_copy(xft_new[0:64, :, bsl],
                                          xff_new[0:64, :, bsl])
                    nc.vector.tensor_copy(xft_new[64:128, :, bsl],
                                          xff_new[64:128, :, bsl])
                nc.sync.dma_start(o_transf[l], xff_new[:])
                xft_bf = xft_new
                xff_cur = xff_new

            # ---- classification head + softmax ----
            pc = psum.tile([BLOC, 10], F32, tag="bank")
            for c in range(32):
                nc.tensor.matmul(pc[:], xff_cur[:, c, :], wctsb[:, c, :],
                                 start=(c == 0), stop=False)
            nc.tensor.matmul(pc[:], ones1sb[:], bc1sb[:], start=False,
                             stop=True)
            cls = work.tile([BLOC, 10], F32, tag="cls")
            nc.vector.tensor_copy(cls[:], pc[:])
            nc.sync.dma_start(o_cls, cls[:])

            mxt = work.tile([BLOC, 1], F32, tag="mx")
            nc.vector.tensor_reduce(mxt[:], cls[:], mybir.AxisListType.X,
                                    mx_op)
            sh = work.tile([BLOC, 10], F32, tag="sh")
            nc.vector.tensor_scalar_sub(sh[:], cls[:], mxt[:])
            ex = work.tile([BLOC, 10], F32, tag="ex")
            nc.scalar.activation(ex[:], sh[:],
                                 mybir.ActivationFunctionType.Exp)
            sm = work.tile([BLOC, 1], F32, tag="sum")
            nc.vector.tensor_reduce(sm[:], ex[:], mybir.AxisListType.X, add)
            nc.vector.reciprocal(sm[:], sm[:])
            prd_t = work.tile([BLOC, 10], F32, tag="pd")
            nc.vector.tensor_scalar_mul(prd_t[:], ex[:], sm[:])
            nc.sync.dma_start(o_pred, prd_t[:])

    nc.compile()
    return nc


def host_prep_shared(W0, W, b, Wc, bc):
    """Per-layer shared tensors (identical on every core)."""
    wq = np.empty((L + 1, 8, 128, 4, 8, 512), dtype=W_NP)
    for l in range(L + 1):
        Wm = W0 if l == 0 else W[l - 1]
        WT = np.ascontiguousarray(Wm.T).astype(W_NP)  # (4096 k, 4096 n)
        # (8r, 4j, 128p, 8c, 512n) -> (r, p, j, c, n)
        wq[l] = WT.reshape(8, 4, 128, 8, 512).transpose(0, 2, 1, 3, 4)
    bt = np.zeros((64, L + 1, 64), dtype=np.float32)
    for l in range(1, L + 1):
        bt[:, l, :] = b[l - 1].reshape(64, 64).T  # [j, i]
    e4 = np.tile(np.eye(BLOC), (4, 1)).astype(X_NP)  # (128, 32)
    wct = np.ascontiguousarray(
        Wc.T.reshape(32, 128, 10).transpose(1, 0, 2)).astype(np.float32)
    bc1 = bc.reshape(1, 10).astype(np.float32)
    ones1 = np.ones((1, BLOC), dtype=np.float32)
    return dict(wq=wq, bt=bt, e4=e4, wct=wct, bc1=bc1, ones1=ones1)


def host_prep_core(Xc):
    """Per-core tensors from this core's 32-sample X slice (32, 3, 1024)."""
    u = Xc[:, 0].reshape(BLOC, 64, 16)
    s = Xc[:, 1].reshape(BLOC, 64, 16)[:, :16, :16]
    vh = Xc[:, 2].reshape(BLOC, 16, 64)
    u64 = np.ascontiguousarray(u.transpose(1, 0, 2)).astype(X_NP)
    ut16 = np.ascontiguousarray(u.transpose(2, 0, 1)).astype(X_NP)
    v64 = np.ascontiguousarray(vh.transpose(2, 0, 1)).astype(X_NP)
    vt16q = np.ascontiguousarray(vh.transpose(1, 0, 2)).astype(X_NP)
    s16 = np.ascontiguousarray(s.transpose(1, 0, 2), dtype=np.float32)
    Xf0 = np.einsum('bik,bkl,blj->bij', u, s, vh,
                    optimize=True).reshape(BLOC, DD)
    # xft0[p, c, b] = Xf0[b, 128c + p]
    xff0 = np.ascontiguousarray(
        Xf0.T.reshape(32, 128, BLOC).transpose(1, 0, 2)).astype(np.float32)
    xft0 = xff0.astype(X_NP)
    return dict(u64=u64, ut16=ut16, v64=v64, vt16q=vt16q,
                xft0=xft0, xff0=xff0)


def assemble_outputs(results):
    """results: list of 8 per-core dicts -> full outputs."""
    preds, clss, transfs = [], [], []
    for r in results:
        preds.append(r["o_pred"])
        clss.append(r["o_cls"])
        ot = r["o_transf"]  # (9, 128, 32, 32) [l, p, c, b]
        transfs.append(np.ascontiguousarray(
            ot.transpose(3, 2, 1, 0)).reshape(BLOC, DD, L + 1))
    X_predicted = np.concatenate(preds, axis=0).astype(np.float32)
    X_classified = np.concatenate(clss, axis=0).astype(np.float32)
    X_transformed = np.concatenate(transfs, axis=0).astype(np.float32)
    return X_predicted, X_classified, X_transformed


def run(X, W0, W, b, Wc, bc, **run_kwargs):
    if "nc" not in _CACHE:
        _CACHE["nc"] = build_nc()
    nc = _CACHE["nc"]
    shared = host_prep_shared(np.asarray(W0, np.float32),
                              np.asarray(W, np.float32),
                              np.asarray(b, np.float32),
                              np.asarray(Wc, np.float32),
                              np.asarray(bc, np.float32))
    X = np.asarray(X, np.float32)
    in_maps = []
    for c in range(NCORES):
        m = dict(shared)
        m.update(host_prep_core(X[c * BLOC:(c + 1) * BLOC]))
        in_maps.append(m)
    res = run_bass_kernel_spmd(nc, in_maps, core_ids=list(range(NCORES)),
                               **run_kwargs)
    return assemble_outputs(res.results), res


def kernel(X, W0, W, b, Wc, bc):
    outs, _ = run(X, W0, W, b, Wc, bc)
    return outs


# revision 15
# speedup vs baseline: 1.9680x; 1.0261x over previous
"""Trainium2 Bass kernel for nn_DynResNet (B=256, DIM=64, K=16, L=8).

Strategy (validated numerically against the jax reference in fp64/fp32):
- Pure data parallel: 32 samples per core x 8 cores; 9 shared 4096x4096
  weights streamed from HBM as fp8e4m3 (weight rounding only affects the
  output through H=1e-3-damped updates; measured end-to-end error ~5e-5).
- The Cayley u/v updates change u and v by ~1e-7 relative (below fp32 ulp
  of u); dropping them is within ~2e-6 of the fp32 reference. Only the s
  update (s += H * u^T relu(lin) v) and Xf = u s v^T recompute remain.
- Big matmul: stationary = Xf^T k-chunks (128, 32) bf16 in 4 column-tiled
  array groups, reused across all 8 output-chunk PSUM banks via
  ldweights=False follow-on matmuls; moving = W^T tiles (128, 512) fp8.
  Partition-group partial sums are reduced AND transposed in one PE pass
  against a stacked identity, yielding dY in the (j, i, b) layout the
  per-sample small matmuls consume.
"""

import numpy as np
import ml_dtypes

import concourse.bass as bass
import concourse.tile as tile
from concourse import bacc, mybir
from concourse.bass_utils import run_bass_kernel_spmd

DIM, KR, L, DD, B, NCORES = 64, 16, 8, 4096, 256, 8
BLOC = B // NCORES  # 32
H = 1e-3
F32 = mybir.dt.float32
W_DT = mybir.dt.float8e4
X_DT = mybir.dt.bfloat16
W_NP = ml_dtypes.float8_e4m3
X_NP = ml_dtypes.bfloat16

_CACHE = {}


def build_nc():
    nc = bacc.Bacc("TRN2", target_bir_lowering=False, debug=False,
                   num_devices=NCORES)

    def inp(name, shape, dt):
        return nc.dram_tensor(name, shape, dt, kind="ExternalInput").ap()

    # wq[l, r, p, j, c, n] = W_l^T[(4r+j)*128 + p, c*512 + n]
    wq = inp("wq", (L + 1, 8, 128, 4, 8, 512), W_DT)
    u64 = inp("u64", (64, BLOC, KR), X_DT)
    ut16 = inp("ut16", (KR, BLOC, 64), X_DT)
    v64 = inp("v64", (64, BLOC, KR), X_DT)
    vt16q = inp("vt16q", (16, BLOC, 64), X_DT)
    xft0 = inp("xft0", (128, 32, BLOC), X_DT)
    xff0 = inp("xff0", (128, 32, BLOC), F32)
    bt = inp("bt", (64, L + 1, 64), F32)
    e4 = inp("e4", (128, BLOC), X_DT)
    wct = inp("wct", (128, 32, 10), F32)
    bc1 = inp("bc1", (1, 10), F32)
    ones1 = inp("ones1", (1, BLOC), F32)

    o_transf = nc.dram_tensor("o_transf", (L + 1, 128, 32, BLOC), F32,
                              kind="ExternalOutput").ap()
    o_cls = nc.dram_tensor("o_cls", (BLOC, 10), F32,
                           kind="ExternalOutput").ap()
    o_pred = nc.dram_tensor("o_pred", (BLOC, 10), F32,
                            kind="ExternalOutput").ap()

    add, mult, mx_op = (mybir.AluOpType.add, mybir.AluOpType.mult,
                        mybir.AluOpType.max)

    with tile.TileContext(nc) as tc:
        with (
            tc.tile_pool(name="consts", bufs=1) as consts,
            tc.tile_pool(name="wpool", bufs=9) as wpool,
            tc.tile_pool(name="xft", bufs=2) as xftp,
            tc.tile_pool(name="xff", bufs=2) as xffp,
            tc.tile_pool(name="dyt", bufs=2) as dytp,
            tc.tile_pool(name="lp", bufs=8) as lpp,
            tc.tile_pool(name="work", bufs=2) as work,
            tc.tile_pool(name="psum", bufs=8, space="PSUM") as psum,
        ):
            def cload(ap_in, shape, dt):
                t = consts.tile(shape, dt, tag=ap_in.tensor.name)
                nc.sync.dma_start(t[:], ap_in)
                return t

            u64sb = cload(u64, [64, BLOC, KR], X_DT)
            ut16sb = cload(ut16, [KR, BLOC, 64], X_DT)
            v64sb = cload(v64, [64, BLOC, KR], X_DT)
            vt16qsb = cload(vt16q, [16, BLOC, 64], X_DT)
            btsb = cload(bt, [64, L + 1, 64], F32)
            e4sb = cload(e4, [128, BLOC], X_DT)
            wctsb = cload(wct, [128, 32, 10], F32)
            bc1sb = cload(bc1, [1, 10], F32)
            ones1sb = cload(ones1, [1, BLOC], F32)

            xft_bf = xftp.tile([128, 32, BLOC], X_DT, tag="xft")
            nc.sync.dma_start(xft_bf[:], xft0)
            xff_cur = xffp.tile([128, 32, BLOC], F32, tag="xff")
            nc.sync.dma_start(xff_cur[:], xff0)

            for l in range(L + 1):
                # ---- big matmul: all 8 n-chunk banks, k-chunk-outer ----
                pa = [psum.tile([128, 512], F32, tag="bank",
                                name=f"pa{l}_{i}")
                      for i in range(8)]
                for r in range(8):
                    wt = wpool.tile([128, 4, 8, 512], W_DT, tag="wt")
                    nc.sync.dma_start(wt[:], wq[l, r])
                    for cn in range(8):
                        for j in range(4):
                            kc = 4 * r + j
                            mm = nc.tensor.matmul(
                                pa[cn][32 * j:32 * j + 32, :],
                                xft_bf[:, kc, :],
                                wt[:, j, cn, :],
                                start=(r == 0), stop=(r == 7),
                                tile_position=(0, 32 * j),
                                skip_group_check=True,
                            )
                            if cn > 0:
                                mm.ldweights = False

                # ---- reduce partition groups + transpose + bias + relu ----
                dyt = dytp.tile([64, 64, BLOC], X_DT, tag="dyt")
                for cn in range(8):
                    lp = lpp.tile([128, 512], X_DT, tag="lp")
                    nc.vector.tensor_copy(lp[:], pa[cn][:])
                    prd = psum.tile([64, 8, BLOC], F32, tag="bank")
                    for m in range(8):
                        nc.tensor.matmul(prd[:, m, :],
                                         lp[:, 64 * m:64 * m + 64],
                                         e4sb[:], start=True, stop=True)
                    dsl = dyt[:, 8 * cn:8 * cn + 8, :]
                    bias_bc = btsb[:, l, 8 * cn:8 * cn + 8][:, :, None] \
                        .broadcast_to((64, 8, BLOC))
                    nc.vector.tensor_add(dsl, prd[:], bias_bc)
                    nc.vector.tensor_scalar_max(dsl, dsl, 0.0)

                # ---- z_u = dY v ; dS = u^T z_u ; s += H dS ----
                pz = psum.tile([64, BLOC, KR], F32, tag="bank")
                for bb in range(BLOC):
                    nc.tensor.matmul(pz[:, bb, :], dyt[:, :, bb],
                                     v64sb[:, bb, :], start=True, stop=True)
                zu = work.tile([64, BLOC, KR], X_DT, tag="zu")
                nc.vector.tensor_copy(zu[:], pz[:])
                pds = psum.tile([KR, BLOC, KR], F32, tag="bank")
                for bb in range(BLOC):
                    nc.tensor.matmul(pds[:, bb, :], u64sb[:, bb, :],
                                     zu[:, bb, :], start=True, stop=True)
                dsb = work.tile([KR, BLOC, KR], X_DT, tag="dsb")
                nc.vector.tensor_scalar_mul(dsb[:], pds[:], float(H))

                # ---- r1 = (H dS)^T u^T per sample ----
                r1 = work.tile([KR, BLOC, 64], X_DT, tag="r1")
                for grp in range(4):
                    p1 = psum.tile([KR, 8, 64], F32, tag="bank")
                    for bb in range(8):
                        bs = grp * 8 + bb
                        nc.tensor.matmul(p1[:, bb, :], dsb[:, bs, :],
                                         ut16sb[:, bs, :], start=True,
                                         stop=True)
                    nc.scalar.activation(r1[:, grp * 8:grp * 8 + 8, :],
                                         p1[:],
                                         mybir.ActivationFunctionType.Copy)

                # ---- dM^T = v r1 ; Xf += dM (both halves), batched ----
                xft_new = xftp.tile([128, 32, BLOC], X_DT, tag="xft")
                xff_new = xffp.tile([128, 32, BLOC], F32, tag="xff")
                for grp in range(4):
                    pm = psum.tile([128, 8, 64], F32, tag="bank")
                    for bb in range(8):
                        bs = grp * 8 + bb
                        vq = vt16qsb[:, bs, :]
                        rq = r1[:, bs, :]
                        nc.tensor.matmul(pm[0:64, bb, :], vq, rq,
                                         start=True, stop=True)
                        nc.tensor.matmul(pm[64:128, bb, :], vq, rq,
                                         start=True, stop=True,
                                         tile_position=(0, 64))
                    bsl = slice(8 * grp, 8 * grp + 8)
                    pm_even = pm[0:64, :, 0:64:2].transpose([0, 2, 1])
                    pm_odd = pm[64:128, :, 1:64:2].transpose([0, 2, 1])
                    nc.vector.tensor_add(xff_new[0:64, :, bsl],
                                         xff_cur[0:64, :, bsl], pm_even)
                    nc.vector.tensor_add(xff_new[64:128, :, bsl],
                                         xff_cur[64:128, :, bsl], pm_odd)
                    nc.vector.tensor_copy(xft_new[0:64, :, bsl],
                                          xff_new[0:64, :, bsl])
                    nc.vector.tensor_copy(xft_new[64:128, :, bsl],
                                          xff_new[64:128, :, bsl])
                nc.sync.dma_start(o_transf[l], xff_new[:])
                xft_bf = xft_new
                xff_cur = xff_new

            # ---- classification head + softmax ----
            pc = psum.tile([BLOC, 10], F32, tag="bank")
            for c in range(32):
                nc.tensor.matmul(pc[:], xff_cur[:, c, :], wctsb[:, c, :],
                                 start=(c == 0), stop=False)
            nc.tensor.matmul(pc[:], ones1sb[:], bc1sb[:], start=False,
                             stop=True)
            cls = work.tile([BLOC, 10], F32, tag="cls")
            nc.vector.tensor_copy(cls[:], pc[:])
            nc.sync.dma_start(o_cls, cls[:])

            mxt = work.tile([BLOC, 1], F32, tag="mx")
            nc.vector.tensor_reduce(mxt[:], cls[:], mybir.AxisListType.X,
                                    mx_op)
            sh = work.tile([BLOC, 10], F32, tag="sh")
            nc.vector.tensor_scalar_sub(sh[:], cls[:], mxt[:])
            ex = work.tile([BLOC, 10], F32, tag="ex")
            nc.scalar.activation(ex[:], sh[:],
                                 mybir.ActivationFunctionType.Exp)
            sm = work.tile([BLOC, 1], F32, tag="sum")
            nc.vector.tensor_reduce(sm[:], ex[:], mybir.AxisListType.X, add)
            nc.vector.reciprocal(sm[:], sm[:])
            prd_t = work.tile([BLOC, 10], F32, tag="pd")
            nc.vector.tensor_scalar_mul(prd_t[:], ex[:], sm[:])
            nc.sync.dma_start(o_pred, prd_t[:])

    nc.compile()
    return nc


def host_prep_shared(W0, W, b, Wc, bc):
    """Per-layer shared tensors (identical on every core)."""
    wq = np.empty((L + 1, 8, 128, 4, 8, 512), dtype=W_NP)
    for l in range(L + 1):
        Wm = W0 if l == 0 else W[l - 1]
        WT = np.ascontiguousarray(Wm.T).astype(W_NP)  # (4096 k, 4096 n)
        # (8r, 4j, 128p, 8c, 512n) -> (r, p, j, c, n)
        wq[l] = WT.reshape(8, 4, 128, 8, 512).transpose(0, 2, 1, 3, 4)
    bt = np.zeros((64, L + 1, 64), dtype=np.float32)
    for l in range(1, L + 1):
        bt[:, l, :] = b[l - 1].reshape(64, 64).T  # [j, i]
    e4 = np.tile(np.eye(BLOC), (4, 1)).astype(X_NP)  # (128, 32)
    wct = np.ascontiguousarray(
        Wc.T.reshape(32, 128, 10).transpose(1, 0, 2)).astype(np.float32)
    bc1 = bc.reshape(1, 10).astype(np.float32)
    ones1 = np.ones((1, BLOC), dtype=np.float32)
    return dict(wq=wq, bt=bt, e4=e4, wct=wct, bc1=bc1, ones1=ones1)


def host_prep_core(Xc):
    """Per-core tensors from this core's 32-sample X slice (32, 3, 1024)."""
    u = Xc[:, 0].reshape(BLOC, 64, 16)
    s = Xc[:, 1].reshape(BLOC, 64, 16)[:, :16, :16]
    vh = Xc[:, 2].reshape(BLOC, 16, 64)
    u64 = np.ascontiguousarray(u.transpose(1, 0, 2)).astype(X_NP)
    ut16 = np.ascontiguousarray(u.transpose(2, 0, 1)).astype(X_NP)
    v64 = np.ascontiguousarray(vh.transpose(2, 0, 1)).astype(X_NP)
    vt16q = np.ascontiguousarray(vh.transpose(1, 0, 2)).astype(X_NP)
    s16 = np.ascontiguousarray(s.transpose(1, 0, 2), dtype=np.float32)
    Xf0 = np.einsum('bik,bkl,blj->bij', u, s, vh,
                    optimize=True).reshape(BLOC, DD)
    # xft0[p, c, b] = Xf0[b, 128c + p]
    xff0 = np.ascontiguousarray(
        Xf0.T.reshape(32, 128, BLOC).transpose(1, 0, 2)).astype(np.float32)
    xft0 = xff0.astype(X_NP)
    return dict(u64=u64, ut16=ut16, v64=v64, vt16q=vt16q,
                xft0=xft0, xff0=xff0)


def assemble_outputs(results):
    """results: list of 8 per-core dicts -> full outputs."""
    preds, clss, transfs = [], [], []
    for r in results:
        preds.append(r["o_pred"])
        clss.append(r["o_cls"])
        ot = r["o_transf"]  # (9, 128, 32, 32) [l, p, c, b]
        transfs.append(np.ascontiguousarray(
            ot.transpose(3, 2, 1, 0)).reshape(BLOC, DD, L + 1))
    X_predicted = np.concatenate(preds, axis=0).astype(np.float32)
    X_classified = np.concatenate(clss, axis=0).astype(np.float32)
    X_transformed = np.concatenate(transfs, axis=0).astype(np.float32)
    return X_predicted, X_classified, X_transformed


def run(X, W0, W, b, Wc, bc, **run_kwargs):
    if "nc" not in _CACHE:
        _CACHE["nc"] = build_nc()
    nc = _CACHE["nc"]
    shared = host_prep_shared(np.asarray(W0, np.float32),
                              np.asarray(W, np.float32),
                              np.asarray(b, np.float32),
                              np.asarray(Wc, np.float32),
                              np.asarray(bc, np.float32))
    X = np.asarray(X, np.float32)
    in_maps = []
    for c in range(NCORES):
        m = dict(shared)
        m.update(host_prep_core(X[c * BLOC:(c + 1) * BLOC]))
        in_maps.append(m)
    res = run_bass_kernel_spmd(nc, in_maps, core_ids=list(range(NCORES)),
                               **run_kwargs)
    return assemble_outputs(res.results), res


def kernel(X, W0, W, b, Wc, bc):
    outs, _ = run(X, W0, W, b, Wc, bc)
    return outs


# revision 16
# speedup vs baseline: 1.9741x; 1.0031x over previous
"""Trainium2 Bass kernel for nn_DynResNet (B=256, DIM=64, K=16, L=8).

Strategy (validated numerically against the jax reference in fp64/fp32):
- Pure data parallel: 32 samples per core x 8 cores; 9 shared 4096x4096
  weights streamed from HBM as fp8e4m3 (weight rounding only affects the
  output through H=1e-3-damped updates; measured end-to-end error ~5e-5).
- The Cayley u/v updates change u and v by ~1e-7 relative (below fp32 ulp
  of u); dropping them is within ~2e-6 of the fp32 reference. Only the s
  update (s += H * u^T relu(lin) v) and Xf = u s v^T recompute remain.
- Big matmul: stationary = Xf^T k-chunks (128, 32) bf16 in 4 column-tiled
  array groups, reused across all 8 output-chunk PSUM banks via
  ldweights=False follow-on matmuls; moving = W^T tiles (128, 512) fp8.
  Partition-group partial sums are reduced AND transposed in one PE pass
  against a stacked identity, yielding dY in the (j, i, b) layout the
  per-sample small matmuls consume.
"""

import numpy as np
import ml_dtypes

import concourse.bass as bass
import concourse.tile as tile
from concourse import bacc, mybir
from concourse.bass_utils import run_bass_kernel_spmd

DIM, KR, L, DD, B, NCORES = 64, 16, 8, 4096, 256, 8
BLOC = B // NCORES  # 32
H = 1e-3
F32 = mybir.dt.float32
W_DT = mybir.dt.float8e4
X_DT = mybir.dt.bfloat16
W_NP = ml_dtypes.float8_e4m3
X_NP = ml_dtypes.bfloat16

_CACHE = {}


def build_nc():
    nc = bacc.Bacc("TRN2", target_bir_lowering=False, debug=False,
                   num_devices=NCORES)

    def inp(name, shape, dt):
        return nc.dram_tensor(name, shape, dt, kind="ExternalInput").ap()

    # wq[l, r, p, j, c, n] = W_l^T[(4r+j)*128 + p, c*512 + n]
    wq = inp("wq", (L + 1, 8, 128, 4, 8, 512), W_DT)
    u64 = inp("u64", (64, BLOC, KR), X_DT)
    ut16 = inp("ut16", (KR, BLOC, 64), X_DT)
    v64 = inp("v64", (64, BLOC, KR), X_DT)
    vt16q = inp("vt16q", (16, BLOC, 64), X_DT)
    xft0 = inp("xft0", (128, 32, BLOC), X_DT)
    xff0 = inp("xff0", (128, 32, BLOC), F32)
    bt = inp("bt", (64, L + 1, 64), F32)
    e4 = inp("e4", (128, BLOC), X_DT)
    wct = inp("wct", (128, 32, 10), F32)
    bc1 = inp("bc1", (1, 10), F32)
    ones1 = inp("ones1", (1, BLOC), F32)

    o_transf = nc.dram_tensor("o_transf", (L + 1, 128, 32, BLOC), F32,
                              kind="ExternalOutput").ap()
    o_cls = nc.dram_tensor("o_cls", (BLOC, 10), F32,
                           kind="ExternalOutput").ap()
    o_pred = nc.dram_tensor("o_pred", (BLOC, 10), F32,
                            kind="ExternalOutput").ap()

    add, mult, mx_op = (mybir.AluOpType.add, mybir.AluOpType.mult,
                        mybir.AluOpType.max)

    with tile.TileContext(nc) as tc:
        with (
            tc.tile_pool(name="consts", bufs=1) as consts,
            tc.tile_pool(name="wpool", bufs=9) as wpool,
            tc.tile_pool(name="xft", bufs=2) as xftp,
            tc.tile_pool(name="xff", bufs=2) as xffp,
            tc.tile_pool(name="dyt", bufs=2) as dytp,
            tc.tile_pool(name="lp", bufs=8) as lpp,
            tc.tile_pool(name="work", bufs=2) as work,
            tc.tile_pool(name="psum", bufs=8, space="PSUM") as psum,
        ):
            def cload(ap_in, shape, dt):
                t = consts.tile(shape, dt, tag=ap_in.tensor.name)
                nc.sync.dma_start(t[:], ap_in)
                return t

            u64sb = cload(u64, [64, BLOC, KR], X_DT)
            ut16sb = cload(ut16, [KR, BLOC, 64], X_DT)
            v64sb = cload(v64, [64, BLOC, KR], X_DT)
            vt16qsb = cload(vt16q, [16, BLOC, 64], X_DT)
            btsb = cload(bt, [64, L + 1, 64], F32)
            e4sb = cload(e4, [128, BLOC], X_DT)
            wctsb = cload(wct, [128, 32, 10], F32)
            bc1sb = cload(bc1, [1, 10], F32)
            ones1sb = cload(ones1, [1, BLOC], F32)

            xft_bf = xftp.tile([128, 32, BLOC], X_DT, tag="xft")
            nc.sync.dma_start(xft_bf[:], xft0)
            xff_cur = xffp.tile([128, 32, BLOC], F32, tag="xff")
            nc.sync.dma_start(xff_cur[:], xff0)

            for l in range(L + 1):
                # ---- big matmul: all 8 n-chunk banks, k-chunk-outer ----
                pa = [psum.tile([128, 512], F32, tag="bank",
                                name=f"pa{l}_{i}")
                      for i in range(8)]
                for r in range(8):
                    wt = wpool.tile([128, 4, 8, 512], W_DT, tag="wt")
                    nc.sync.dma_start(wt[:], wq[l, r])
                    for cn in range(8):
                        for j in range(4):
                            kc = 4 * r + j
                            mm = nc.tensor.matmul(
                                pa[cn][32 * j:32 * j + 32, :],
                                xft_bf[:, kc, :],
                                wt[:, j, cn, :],
                                start=(r == 0), stop=(r == 7),
                                tile_position=(0, 32 * j),
                                skip_group_check=True,
                            )
                            if cn > 0:
                                mm.ldweights = False

                # ---- reduce partition groups + transpose + bias + relu ----
                dyt = dytp.tile([64, 64, BLOC], X_DT, tag="dyt")
                for cn in range(8):
                    lp = lpp.tile([128, 512], X_DT, tag="lp")
                    nc.vector.tensor_copy(lp[:], pa[cn][:])
                    prd = psum.tile([64, 8, BLOC], F32, tag="bank")
                    for m in range(8):
                        nc.tensor.matmul(prd[:, m, :],
                                         lp[:, 64 * m:64 * m + 64],
                                         e4sb[:], start=True, stop=True)
                    dsl = dyt[:, 8 * cn:8 * cn + 8, :]
                    bias_bc = btsb[:, l, 8 * cn:8 * cn + 8][:, :, None] \
                        .broadcast_to((64, 8, BLOC))
                    nc.vector.tensor_add(dsl, prd[:], bias_bc)
                    nc.vector.tensor_scalar_max(dsl, dsl, 0.0)

                # ---- z_u = dY v ; dS = u^T z_u ; s += H dS ----
                pz = psum.tile([64, BLOC, KR], F32, tag="bank")
                for bb in range(BLOC):
                    nc.tensor.matmul(pz[:, bb, :], dyt[:, :, bb],
                                     v64sb[:, bb, :], start=True, stop=True)
                zu = work.tile([64, BLOC, KR], X_DT, tag="zu")
                nc.vector.tensor_copy(zu[:], pz[:])
                pds = psum.tile([KR, BLOC, KR], F32, tag="bank")
                for bb in range(BLOC):
                    nc.tensor.matmul(pds[:, bb, :], u64sb[:, bb, :],
                                     zu[:, bb, :], start=True, stop=True)
                dsb = work.tile([KR, BLOC, KR], X_DT, tag="dsb")
                nc.vector.tensor_scalar_mul(dsb[:], pds[:], float(H))

                # ---- r1 = (H dS)^T u^T per sample ----
                r1 = work.tile([KR, BLOC, 64], X_DT, tag="r1")
                for grp in range(4):
                    p1 = psum.tile([KR, 8, 64], F32, tag="bank")
                    for bb in range(8):
                        bs = grp * 8 + bb
                        nc.tensor.matmul(p1[:, bb, :], dsb[:, bs, :],
                                         ut16sb[:, bs, :], start=True,
                                         stop=True)
                    nc.scalar.activation(r1[:, grp * 8:grp * 8 + 8, :],
                                         p1[:],
                                         mybir.ActivationFunctionType.Copy)

                # ---- dM^T = v r1 ; Xf += dM (both halves), batched ----
                xft_new = xftp.tile([128, 32, BLOC], X_DT, tag="xft")
                xff_new = xffp.tile([128, 32, BLOC], F32, tag="xff")
                for grp in range(4):
                    pm = psum.tile([128, 8, 64], F32, tag="bank")
                    for bb in range(8):
                        bs = grp * 8 + bb
                        vq = vt16qsb[:, bs, :]
                        rq = r1[:, bs, :]
                        nc.tensor.matmul(pm[0:64, bb, :], vq, rq,
                                         start=True, stop=True)
                        nc.tensor.matmul(pm[64:128, bb, :], vq, rq,
                                         start=True, stop=True,
                                         tile_position=(0, 64))
                    bsl = slice(8 * grp, 8 * grp + 8)
                    pm_even = pm[0:64, :, 0:64:2].transpose([0, 2, 1])
                    pm_odd = pm[64:128, :, 1:64:2].transpose([0, 2, 1])
                    nc.vector.tensor_add(xff_new[0:64, :, bsl],
                                         xff_cur[0:64, :, bsl], pm_even)
                    nc.vector.tensor_add(xff_new[64:128, :, bsl],
                                         xff_cur[64:128, :, bsl], pm_odd)
                    nc.scalar.activation(xft_new[0:64, :, bsl],
                                         xff_new[0:64, :, bsl],
                                         mybir.ActivationFunctionType.Copy)
                    nc.scalar.activation(xft_new[64:128, :, bsl],
                                         xff_new[64:128, :, bsl],
                                         mybir.ActivationFunctionType.Copy)
                nc.sync.dma_start(o_transf[l], xff_new[:])
                xft_bf = xft_new
                xff_cur = xff_new

            # ---- classification head + softmax ----
            pc = psum.tile([BLOC, 10], F32, tag="bank")
            for c in range(32):
                nc.tensor.matmul(pc[:], xff_cur[:, c, :], wctsb[:, c, :],
                                 start=(c == 0), stop=False)
            nc.tensor.matmul(pc[:], ones1sb[:], bc1sb[:], start=False,
                             stop=True)
            cls = work.tile([BLOC, 10], F32, tag="cls")
            nc.vector.tensor_copy(cls[:], pc[:])
            nc.sync.dma_start(o_cls, cls[:])

            mxt = work.tile([BLOC, 1], F32, tag="mx")
            nc.vector.tensor_reduce(mxt[:], cls[:], mybir.AxisListType.X,
                                    mx_op)
            sh = work.tile([BLOC, 10], F32, tag="sh")
            nc.vector.tensor_scalar_sub(sh[:], cls[:], mxt[:])
            ex = work.tile([BLOC, 10], F32, tag="ex")
            nc.scalar.activation(ex[:], sh[:],
                                 mybir.ActivationFunctionType.Exp)
            sm = work.tile([BLOC, 1], F32, tag="sum")
            nc.vector.tensor_reduce(sm[:], ex[:], mybir.AxisListType.X, add)
            nc.vector.reciprocal(sm[:], sm[:])
            prd_t = work.tile([BLOC, 10], F32, tag="pd")
            nc.vector.tensor_scalar_mul(prd_t[:], ex[:], sm[:])
            nc.sync.dma_start(o_pred, prd_t[:])

    nc.compile()
    return nc


def host_prep_shared(W0, W, b, Wc, bc):
    """Per-layer shared tensors (identical on every core)."""
    wq = np.empty((L + 1, 8, 128, 4, 8, 512), dtype=W_NP)
    for l in range(L + 1):
        Wm = W0 if l == 0 else W[l - 1]
        WT = np.ascontiguousarray(Wm.T).astype(W_NP)  # (4096 k, 4096 n)
        # (8r, 4j, 128p, 8c, 512n) -> (r, p, j, c, n)
        wq[l] = WT.reshape(8, 4, 128, 8, 512).transpose(0, 2, 1, 3, 4)
    bt = np.zeros((64, L + 1, 64), dtype=np.float32)
    for l in range(1, L + 1):
        bt[:, l, :] = b[l - 1].reshape(64, 64).T  # [j, i]
    e4 = np.tile(np.eye(BLOC), (4, 1)).astype(X_NP)  # (128, 32)
    wct = np.ascontiguousarray(
        Wc.T.reshape(32, 128, 10).transpose(1, 0, 2)).astype(np.float32)
    bc1 = bc.reshape(1, 10).astype(np.float32)
    ones1 = np.ones((1, BLOC), dtype=np.float32)
    return dict(wq=wq, bt=bt, e4=e4, wct=wct, bc1=bc1, ones1=ones1)


def host_prep_core(Xc):
    """Per-core tensors from this core's 32-sample X slice (32, 3, 1024)."""
    u = Xc[:, 0].reshape(BLOC, 64, 16)
    s = Xc[:, 1].reshape(BLOC, 64, 16)[:, :16, :16]
    vh = Xc[:, 2].reshape(BLOC, 16, 64)
    u64 = np.ascontiguousarray(u.transpose(1, 0, 2)).astype(X_NP)
    ut16 = np.ascontiguousarray(u.transpose(2, 0, 1)).astype(X_NP)
    v64 = np.ascontiguousarray(vh.transpose(2, 0, 1)).astype(X_NP)
    vt16q = np.ascontiguousarray(vh.transpose(1, 0, 2)).astype(X_NP)
    s16 = np.ascontiguousarray(s.transpose(1, 0, 2), dtype=np.float32)
    Xf0 = np.einsum('bik,bkl,blj->bij', u, s, vh,
                    optimize=True).reshape(BLOC, DD)
    # xft0[p, c, b] = Xf0[b, 128c + p]
    xff0 = np.ascontiguousarray(
        Xf0.T.reshape(32, 128, BLOC).transpose(1, 0, 2)).astype(np.float32)
    xft0 = xff0.astype(X_NP)
    return dict(u64=u64, ut16=ut16, v64=v64, vt16q=vt16q,
                xft0=xft0, xff0=xff0)


def assemble_outputs(results):
    """results: list of 8 per-core dicts -> full outputs."""
    preds, clss, transfs = [], [], []
    for r in results:
        preds.append(r["o_pred"])
        clss.append(r["o_cls"])
        ot = r["o_transf"]  # (9, 128, 32, 32) [l, p, c, b]
        transfs.append(np.ascontiguousarray(
            ot.transpose(3, 2, 1, 0)).reshape(BLOC, DD, L + 1))
    X_predicted = np.concatenate(preds, axis=0).astype(np.float32)
    X_classified = np.concatenate(clss, axis=0).astype(np.float32)
    X_transformed = np.concatenate(transfs, axis=0).astype(np.float32)
    return X_predicted, X_classified, X_transformed


def run(X, W0, W, b, Wc, bc, **run_kwargs):
    if "nc" not in _CACHE:
        _CACHE["nc"] = build_nc()
    nc = _CACHE["nc"]
    shared = host_prep_shared(np.asarray(W0, np.float32),
                              np.asarray(W, np.float32),
                              np.asarray(b, np.float32),
                              np.asarray(Wc, np.float32),
                              np.asarray(bc, np.float32))
    X = np.asarray(X, np.float32)
    in_maps = []
    for c in range(NCORES):
        m = dict(shared)
        m.update(host_prep_core(X[c * BLOC:(c + 1) * BLOC]))
        in_maps.append(m)
    res = run_bass_kernel_spmd(nc, in_maps, core_ids=list(range(NCORES)),
                               **run_kwargs)
    return assemble_outputs(res.results), res


def kernel(X, W0, W, b, Wc, bc):
    outs, _ = run(X, W0, W, b, Wc, bc)
    return outs
